# revision 60
# baseline (speedup 1.0000x reference)
"""Causal self-attention (B=4, T=2048, C=768, 12 heads) on 8 trn2 NeuronCores.

Sharding: core c handles batch b = c//2 and head-group hg = c%2 (6 heads each).
Each core computes its 6 heads end-to-end plus its slice of the output
projection; the two head-group partial projections per batch are summed on the
host (one 6 MB add per batch).

Per-core pipeline (matmuls in bf16 with fp32 PSUM accumulation; softmax fp32):
  - x tiles cast to bf16 on host, transposed to xT; first chunk's DMA split
    per t-tile so the V matmuls start as soon as possible
  - qT/kT = W_qk^T x^T via PE (heads pair-packed into 128-partition tiles)
  - V in natural [t, hd] layout via PE with xT as the stationary operand,
    with a ones column appended per head for the softmax denominator
  - S^T[k,q] = K Q^T per head, two heads per PE slot via tile_position row
    packing (contraction dim = hd = 64); diagonal blocks compute only the
    live column range
  - exp on ACT straight out of PSUM (two heads per op) -> bf16 E; causal
    zero-fill via a host-built triangular bf16 mask multiplied on DVE
  - PV accumulation in PSUM over k tiles (M=65: 64 value dims + denominator)
  - normalization: den + raw-y staged out of PSUM immediately, den row DMA'd
    to partition 0, DVE reciprocal, GpSimd partition_broadcast, muls; the
    bcast+muls of each head pair are deferred one pair

Scheduling (v2): the exp load is strongly back-weighted (the last q-chunk has
5x the exp columns of the first) while its attention matmul work is tiny, so
phase-sequential emission starves either PE or ACT. Instead:
  - the A-phase (QKV) chains of chunk qc+1 and the projection of chunk qc-1
    are interleaved as PE filler UNITS between attention k-blocks (one eager
    filler at each head-pair boundary where S waits on the exp backlog);
  - A-phase PSUM->SBUF staging rides on ACT for early chunks (idle there)
    and on DVE for chunk 3 (ACT is exp-saturated by then);
  - DRAM inputs are host-prearranged partition-major so DMAs move multi-KB
    contiguous lines; wv/x chunk 0 are split so the first chains start early;
  - the final head pair is normalized per output QUARTER as each diagonal
    block's PV lands (quarter j is final after diag block j), reading raw y
    straight from PSUM, with the final projection t-tiles pipelined behind
    the quarter chains -- no keep-warm bridge needed at the tail.
"""

import ml_dtypes
import numpy as np

import concourse.bacc as bacc
import concourse.mybir as mybir
import concourse.tile as tile
from concourse.bass_utils import run_bass_kernel_spmd

F32 = mybir.dt.float32
BF16 = mybir.dt.bfloat16
AF = mybir.ActivationFunctionType

B, T, C = 4, 2048, 768
NH, HD = 12, 64
TC = 4  # t-chunks of 512
CC = 6  # contraction chunks of 128 over C
N_TT = 16  # t tiles of 128

_nc_cache = {}


def _build(with_bias: bool):
    nc = bacc.Bacc(None, target_bir_lowering=False)
    # host-prearranged layouts: partition dim first, contiguous per-partition
    # lines so the DRAM->SBUF DMAs move KB-sized packets
    xt_d = nc.dram_tensor("xt", [TC, 128, CC, 512], BF16, kind="ExternalInput")
    wqk = nc.dram_tensor("wqk", [128, CC, 768], BF16, kind="ExternalInput")
    wv = nc.dram_tensor("wv", [128, CC, 384], BF16, kind="ExternalInput")
    wp = nc.dram_tensor("wp", [128, 3, 768], BF16, kind="ExternalInput")
    msk = nc.dram_tensor("msk", [128, 256], BF16, kind="ExternalInput")
    if with_bias:
        bqk = nc.dram_tensor("bqk", [1, 768], BF16, kind="ExternalInput")
        bv = nc.dram_tensor("bv", [1, 384], BF16, kind="ExternalInput")
    out = nc.dram_tensor("out", [T, C], F32, kind="ExternalOutput")

    with tile.TileContext(nc) as tc, nc.allow_low_precision(
        reason="bf16 matmul operands are intentional"
    ):
        with (
            tc.tile_pool(name="const", bufs=1) as const,
            tc.tile_pool(name="xt", bufs=2) as xtp,
            tc.tile_pool(name="big", bufs=1) as big,
            tc.tile_pool(name="E", bufs=4) as epool,
            tc.tile_pool(name="norm", bufs=2) as npool,
            tc.tile_pool(name="ost", bufs=3) as opool,
            tc.tile_pool(name="ps", bufs=1, space="PSUM") as ps,
        ):
            # ---------------- constants ----------------
            # wdum memset first: it gates the PE warmup matmuls
            wdum = const.tile([128, 512], BF16, name="wdum")
            nc.vector.memset(wdum.bitcast(F32)[:, 0:256], 0.0)

            onecol_f = const.tile([128, 8], F32)
            nc.vector.memset(onecol_f, 1.0)
            onecol_bf = const.tile([128, 8], BF16)
            nc.vector.tensor_copy(onecol_bf, onecol_f)

            # warm the ACT exp table while DMAs run
            warm_f = const.tile([1, 8], F32)
            nc.scalar.activation(warm_f, onecol_f[0:1, :], AF.Exp, scale=1.0)

            if with_bias:
                ones_f = const.tile([1, 512], F32)
                nc.vector.memset(ones_f, 1.0)
                ones_bf = const.tile([1, 512], BF16)
                nc.vector.tensor_copy(ones_bf, ones_f)

            ones64_f = const.tile([1, 64], F32)
            nc.vector.memset(ones64_f, 1.0)

            # PE warmup: dummy matmuls with no input deps keep the HAM
            # activity monitor busy while the first DMAs land; long enough
            # that the clock gate opens (~3.4us) BEFORE the first real A(0)
            # chains, which otherwise run at half clock
            wps = ps.tile([128, 512], F32, tag="mm", bufs=2, name="warmps")
            for _ in range(14):
                nc.tensor.matmul(wps, wdum[:, 0:128], wdum, start=True, stop=True)
            warm_sb = const.tile([1, 8], F32, name="warmsb")
            nc.vector.tensor_copy(warm_sb, wps[0:1, 0:8])

            # ---------------- weights (bf16, host-prepared) ----------------
            # wv split per cc-pair so the first V chain starts after ~1/3
            wv_t = const.tile([128, CC, 384], BF16, name="wv_t")
            for ccp in range(3):
                nc.sync.dma_start(
                    out=wv_t[:, 2 * ccp : 2 * ccp + 2, :],
                    in_=wv[:, 2 * ccp : 2 * ccp + 2, :],
                )
            msk_sb = const.tile([128, 2, 128], BF16, name="msk_sb")
            wv_bf = [wv_t[:, cc, :] for cc in range(CC)]
            wqk_t = const.tile([128, CC, 768], BF16, name="wqk_t")
            wp_t = const.tile([128, 3, 768], BF16, name="wp_t")
            if with_bias:
                bqk_bf = const.tile([1, 768], BF16)
                bv_bf = const.tile([1, 384], BF16)

            # persistent big tiles
            qkT = [big.tile([128, T], BF16, name=f"qkT{ct}") for ct in range(6)]
            v_sb = [big.tile([128, 390], BF16, name=f"v{tt}") for tt in range(N_TT)]
            yT3 = [big.tile([128, T], BF16, name=f"yT{hp}") for hp in range(3)]

            # the denominator ones-columns of v_sb are constant: prefill once
            for tt in range(N_TT):
                vv = v_sb[tt].rearrange("p (h w) -> p h w", w=65)
                nc.vector.tensor_copy(vv[:, :, 64], onecol_bf[:, 0:6])

            def emit_xt_dma(qc):
                """Prefetch the x chunk for chunk qc; returns the tile."""
                xt_t = xtp.tile([128, CC, 512], BF16, tag="xt", name=f"xt_{qc}")
                if qc == 0:
                    # split the first chunk's DMA per contraction chunk so the
                    # first V/qk chains can start as soon as cc=0 lands
                    for cc in range(CC):
                        nc.sync.dma_start(
                            out=xt_t[:, cc, :], in_=xt_d[qc, :, cc, :]
                        )
                else:
                    nc.sync.dma_start(out=xt_t, in_=xt_d[qc, :, :, :])
                return xt_t

            def emit_v_unit(qc, xt_t, tt4):
                """V (+denominator ones) for one t-tile of chunk qc."""
                tt = qc * 4 + tt4
                xt_tiles = [xt_t[:, cc, :] for cc in range(CC)]
                v_ps = ps.tile([128, 384], F32, tag="mm", bufs=2, name=f"vps{tt}")
                for cc in range(CC):
                    nc.tensor.matmul(
                        v_ps,
                        xt_tiles[cc][:, tt4 * 128 : (tt4 + 1) * 128],
                        wv_bf[cc],
                        start=(cc == 0),
                        stop=(cc == CC - 1 and not with_bias),
                    )
                if with_bias:
                    nc.tensor.matmul(
                        v_ps, ones_bf[:, 0:128], bv_bf, start=False, stop=True
                    )
                vv = v_sb[tt].rearrange("p (h w) -> p h w", w=65)
                # early chunks' staging rides on ACT (idle then); chunk 3's
                # lands mid/late attention where ACT is exp-saturated -> DVE
                if qc >= 3:
                    nc.vector.tensor_copy(
                        vv[:, :, 0:64], v_ps.rearrange("p (h w) -> p h w", w=64)
                    )
                else:
                    nc.scalar.copy(
                        vv[:, :, 0:64], v_ps.rearrange("p (h w) -> p h w", w=64)
                    )

            def emit_qk_unit(qc, xt_t, ct):
                """One 128-col slice of qT/kT for chunk qc."""
                xt_tiles = [xt_t[:, cc, :] for cc in range(CC)]
                wqk_bf = [wqk_t[:, cc, :] for cc in range(CC)]
                qk_ps = ps.tile(
                    [128, 512], F32, tag="mm", bufs=2, name=f"qkps{qc}_{ct}"
                )
                for cc in range(CC):
                    nc.tensor.matmul(
                        qk_ps,
                        wqk_bf[cc][:, ct * 128 : (ct + 1) * 128],
                        xt_tiles[cc],
                        start=(cc == 0),
                        stop=(cc == CC - 1 and not with_bias),
                    )
                if with_bias:
                    nc.tensor.matmul(
                        qk_ps,
                        bqk_bf[:, ct * 128 : (ct + 1) * 128],
                        ones_bf,
                        start=False,
                        stop=True,
                    )
                if qc >= 3:
                    nc.vector.tensor_copy(
                        qkT[ct][:, qc * 512 : (qc + 1) * 512], qk_ps
                    )
                else:
                    nc.scalar.copy(qkT[ct][:, qc * 512 : (qc + 1) * 512], qk_ps)

            def emit_proj_unit(qc, tt, split_last=False):
                """Output projection + store for one t-tile of chunk qc.

                With split_last, the hp=2 contribution is emitted as two
                row-tiled halves so the lower half (written directly by the
                norm mul) does not wait for the partition-shift DMA of the
                upper half — shortens the critical tail chain.
                """
                t_sl = slice(tt * 128, (tt + 1) * 128)
                ostage = opool.tile([128, 768], F32, tag="ost")
                for half in range(2):
                    n_sl = slice(half * 384, (half + 1) * 384)
                    pp = ps.tile(
                        [128, 384], F32, tag="mm", bufs=2, name=f"pj{tt}_{half}"
                    )
                    for hp in range(2):
                        nc.tensor.matmul(
                            pp,
                            yT3[hp][:, t_sl],
                            wp_t[:, hp, n_sl],
                            start=(hp == 0),
                            stop=False,
                        )
                    if split_last:
                        nc.tensor.matmul(
                            pp,
                            yT3[2][0:64, t_sl],
                            wp_t[0:64, 2, n_sl],
                            start=False,
                            stop=False,
                            tile_position=(0, 0),
                        )
                        nc.tensor.matmul(
                            pp,
                            yT3[2][64:128, t_sl],
                            wp_t[64:128, 2, n_sl],
                            start=False,
                            stop=True,
                            tile_position=(64, 0),
                        )
                    else:
                        nc.tensor.matmul(
                            pp,
                            yT3[2][:, t_sl],
                            wp_t[:, 2, n_sl],
                            start=False,
                            stop=True,
                        )
                    nc.vector.tensor_copy(ostage[:, n_sl], pp)
                nc.sync.dma_start(out=out[t_sl, :], in_=ostage)

            def emit_attention(qc, fillers):
                """Attention for chunk qc with PE filler units interleaved
                between k-blocks. Returns (last_yst, pending_finishes)."""
                q_sl = slice(qc * 512, (qc + 1) * 512)
                n_kt = 4 * qc + 4
                nblocks = 3 * n_kt
                nfill = len(fillers)
                bi = 0
                fi = 0

                def tick():
                    # advance the filler schedule after each k-block
                    nonlocal bi, fi
                    bi += 1
                    while fi < nfill and fi * nblocks < bi * nfill:
                        fillers[fi]()
                        fi += 1

                tail_quarters = []

                def _tail_quarter_p1(j, yT_a, yT_b):
                    """Phase 1 of the final head pair's quarter-j norm chain
                    (den row staging + DMA to partition 0). Emitted right
                    after diag block j's PV: only cheap wait-free DVE ops, so
                    the remaining attention blocks are not stalled. Phase 2
                    (DMA-dependent) is deferred past the kt loop."""
                    qsl_j = slice(j * 128, (j + 1) * 128)
                    denq = npool.tile([128, 256], F32, tag="denq", bufs=4)
                    nc.vector.tensor_copy(denq[64:65, 0:128], yT_a[64:65, qsl_j])
                    nc.vector.tensor_copy(denq[64:65, 128:256], yT_b[64:65, qsl_j])
                    d0q = npool.tile([1, 512], F32, tag="d0q", bufs=4)
                    nc.sync.dma_start(out=d0q[0:1, 0:256], in_=denq[64:65, :])
                    tail_quarters.append((j, d0q, yT_a, yT_b))

                def _tail_quarter_p2(j, d0q, yT_a, yT_b):
                    """Phase 2: reciprocal, broadcast, PSUM-direct muls (all
                    base-0 operands), partition-shift DMA. The projection
                    t-tiles are emitted separately so the PE queue order
                    keeps all attention matmuls first."""
                    qsl_j = slice(j * 128, (j + 1) * 128)
                    q_abs = slice(qc * 512 + j * 128, qc * 512 + (j + 1) * 128)
                    nc.vector.reciprocal_approx_fast(
                        d0q[0:1, 256:512], d0q[0:1, 0:256]
                    )
                    recq = npool.tile([64, 256], F32, tag="recq", bufs=4)
                    nc.gpsimd.partition_broadcast(
                        recq, d0q[0:1, 256:512], channels=64
                    )
                    nc.vector.tensor_mul(
                        yT3[2][0:64, q_abs], yT_a[0:64, qsl_j], recq[:, 0:128]
                    )
                    ytq = npool.tile([64, 128], BF16, tag="ytq", bufs=4)
                    nc.vector.tensor_mul(ytq, yT_b[0:64, qsl_j], recq[:, 128:256])
                    nc.sync.dma_start(out=yT3[2][64:128, q_abs], in_=ytq)

                pending = []
                last_den = None
                for hp in range(3):
                    if fi < nfill:
                        # one eager filler at each head-pair boundary: the new
                        # pair's first S waits on the previous pair's exp
                        # backlog, so give PE something to chew on
                        fillers[fi]()
                        fi += 1
                    yT_a = ps.tile([65, 512], F32, tag="yT", bufs=2, name=f"ya{qc}_{hp}")
                    yT_b = ps.tile([65, 512], F32, tag="yT", bufs=2, name=f"yb{qc}_{hp}")
                    e_hist = []
                    for kt in range(n_kt):
                        k_sl = slice(kt * 128, (kt + 1) * 128)
                        m = kt - 4 * qc
                        diag = m >= 0
                        w = 512 - 128 * max(m, 0)  # live column range
                        f0 = 512 - w
                        psS = ps.tile(
                            [128, 1024], F32, tag="S", bufs=2, name=f"s{qc}_{hp}_{kt}"
                        )
                        nc.tensor.matmul(
                            psS[:, f0:512],
                            qkT[3 + hp][0:64, k_sl],
                            qkT[hp][0:64, qc * 512 + f0 : (qc + 1) * 512],
                            start=True,
                            stop=True,
                            tile_position=(0, 0),
                        )
                        nc.tensor.matmul(
                            psS[:, 512 + f0 : 1024],
                            qkT[3 + hp][64:128, k_sl],
                            qkT[hp][64:128, qc * 512 + f0 : (qc + 1) * 512],
                            start=True,
                            stop=True,
                            tile_position=(64, 0),
                        )
                        E = epool.tile(
                            [128, 1024], BF16, tag="E", bufs=8, name=f"e{qc}_{hp}_{kt}"
                        )
                        psv = psS.rearrange("p (h w) -> p h w", w=512)
                        ev = E.rearrange("p (h w) -> p h w", w=512)
                        nc.scalar.activation(
                            ev[:, :, f0:512], psv[:, :, f0:512], AF.Exp, scale=0.125
                        )
                        if diag:
                            # causal zero-fill via host-built triangular mask
                            # (DVE: GpSimd's dispatch latency stalls the
                            # E->PV chain even though it has idle capacity)
                            nc.vector.tensor_mul(
                                ev[:, :, f0 : f0 + 128],
                                ev[:, :, f0 : f0 + 128],
                                msk_sb,
                            )
                        e_hist.append((kt, E, f0))
                        tail_pair = qc == TC - 1 and hp == 2
                        if tail_pair and kt >= 4 * qc:
                            # tail head pair, diagonal region: drain PV
                            # eagerly; quarter j of the output columns is
                            # final once diag block m=j has accumulated, so
                            # its normalization chain and its projection
                            # t-tile can start while attention still runs
                            while e_hist:
                                _pv(nc, v_sb, yT_a, yT_b, hp, *e_hist.pop(0), n_kt)
                            if pending:
                                pending.pop(0)()
                            _tail_quarter_p1(kt - 4 * qc, yT_a, yT_b)
                            _tail_quarter_p2(*tail_quarters.pop(0))
                            emit_proj_unit(qc, kt)
                        elif len(e_hist) > 3:
                            # drain two blocks at once: fewer S<->PV
                            # transitions means fewer exposed PV weight loads
                            _pv(nc, v_sb, yT_a, yT_b, hp, *e_hist.pop(0), n_kt)
                            _pv(nc, v_sb, yT_a, yT_b, hp, *e_hist.pop(0), n_kt)
                        tick()
                    if qc == TC - 1 and hp == 2:
                        continue  # tail handled per-quarter above
                    while e_hist:
                        _pv(nc, v_sb, yT_a, yT_b, hp, *e_hist.pop(0), n_kt)

                    # stage denominator + raw y out of PSUM immediately (frees
                    # the yT banks); den row DMA'd to partition 0 (DVE recip
                    # ucode requires base 0). The broadcast + muls are
                    # deferred one head pair.
                    den = npool.tile(
                        [128, 1024], F32, tag="den", bufs=2, name=f"dn{qc}_{hp}"
                    )
                    nc.vector.tensor_copy(den[64:65, 0:512], yT_a[64:65, :])
                    nc.vector.tensor_copy(den[64:65, 512:1024], yT_b[64:65, :])
                    last_den = den
                    yst = npool.tile(
                        [64, 1024], BF16, tag="yst", bufs=3, name=f"ys{qc}_{hp}"
                    )
                    nc.vector.tensor_copy(yst[:, 0:512], yT_a[0:64, :])
                    nc.vector.tensor_copy(yst[:, 512:1024], yT_b[0:64, :])
                    den0 = npool.tile(
                        [1, 2048], F32, tag="den0", bufs=3, name=f"d0{qc}_{hp}"
                    )
                    nc.sync.dma_start(out=den0[0:1, 0:1024], in_=den[64:65, :])

                    def finish(hp=hp, yst=yst, den0=den0):
                        nc.vector.reciprocal_approx_fast(
                            den0[0:1, 1024:2048], den0[0:1, 0:1024]
                        )
                        rec = npool.tile(
                            [64, 1024], F32, tag="rec", bufs=3, name=f"rc{qc}_{hp}"
                        )
                        nc.gpsimd.partition_broadcast(
                            rec, den0[0:1, 1024:2048], channels=64
                        )
                        nc.vector.tensor_mul(
                            yT3[hp][0:64, q_sl], yst[:, 0:512], rec[:, 0:512]
                        )
                        ytmp = npool.tile(
                            [64, 512], BF16, tag="ytmp", bufs=2, name=f"yt{qc}_{hp}"
                        )
                        nc.vector.tensor_mul(ytmp, yst[:, 512:1024], rec[:, 512:1024])
                        # partition shift 0:64 -> 64:128 via SBUF->SBUF DMA
                        nc.sync.dma_start(out=yT3[hp][64:128, q_sl], in_=ytmp)

                    if pending:
                        pending.pop(0)()
                    pending.append(finish)
                while fi < nfill:
                    fillers[fi]()
                    fi += 1
                return last_den, pending

            # ---------------- main schedule ----------------
            # A(0) emitted directly (nothing to interleave with); for qc>=0
            # attention(qc) interleaves the A-chains of qc+1 and the
            # projection of qc-1 as PE filler units.
            xt0 = emit_xt_dma(0)
            nc.sync.dma_start(out=wqk_t[:, 0:3, :], in_=wqk[:, 0:3, :])
            nc.sync.dma_start(out=wqk_t[:, 3:6, :], in_=wqk[:, 3:6, :])
            nc.sync.dma_start(
                out=msk_sb, in_=msk.rearrange("p (h w) -> p h w", w=128)
            )
            if with_bias:
                nc.sync.dma_start(out=bqk_bf, in_=bqk[:, :])
                nc.sync.dma_start(out=bv_bf, in_=bv[:, :])
            for tt4 in range(4):
                emit_v_unit(0, xt0, tt4)
            for ct in range(6):
                emit_qk_unit(0, xt0, ct)
            nc.sync.dma_start(out=wp_t, in_=wp[:, :, :])

            pend_prev = []
            xt_next = None
            attn_state = {}
            for qc in range(TC):
                if qc < TC - 1:
                    xt_next = emit_xt_dma(qc + 1)
                    xt_cap = xt_next

                    def mk_v(tt4, xt_cap=xt_cap, q1=qc + 1):
                        return lambda: emit_v_unit(q1, xt_cap, tt4)

                    def mk_qk(ct, xt_cap=xt_cap, q1=qc + 1):
                        return lambda: emit_qk_unit(q1, xt_cap, ct)

                    aunits = [mk_v(tt4) for tt4 in range(4)] + [
                        mk_qk(ct) for ct in range(6)
                    ]
                else:
                    aunits = []
                if qc >= 1:
                    punits = [
                        (lambda tt=tt: emit_proj_unit(qc - 1, tt))
                        for tt in range((qc - 1) * 4, (qc - 1) * 4 + 4)
                    ]
                else:
                    punits = []
                # filler order: two A-units, then the previous chunk's
                # deferred norm finish (its den0 DMA needs latency headroom,
                # but it must precede the proj units), then alternate the
                # remaining A-units with the proj units
                fillers = aunits[:2] + list(pend_prev)
                rest_a = aunits[2:]
                ai, pi = 0, 0
                for i in range(len(rest_a) + len(punits)):
                    if ai < len(rest_a) and (pi >= len(punits) or i % 2 == 0):
                        fillers.append(rest_a[ai])
                        ai += 1
                    else:
                        fillers.append(punits[pi])
                        pi += 1
                last_den, pend_prev = emit_attention(qc, fillers)
            # tail norm chains + final projection are emitted inside the last
            # chunk's attention (quarter-pipelined per diagonal block)

    nc.finalize()
    return nc


def _pv(nc, v_sb, yT_a, yT_b, hp, kt, E, f0, n_kt):
    a = 2 * hp
    nc.tensor.matmul(
        yT_a[:, f0:512],
        v_sb[kt][:, a * 65 : (a + 1) * 65],
        E[:, f0:512],
        start=(kt == 0),
        stop=(kt == n_kt - 1),
    )
    nc.tensor.matmul(
        yT_b[:, f0:512],
        v_sb[kt][:, (a + 1) * 65 : (a + 2) * 65],
        E[:, 512 + f0 : 1024],
        start=(kt == 0),
        stop=(kt == n_kt - 1),
    )


def _get_nc(with_bias: bool):
    if with_bias not in _nc_cache:
        _nc_cache[with_bias] = _build(with_bias)
    return _nc_cache[with_bias]


def kernel(x, W_attn, b_attn, W_proj, b_proj, _run_kwargs=None):
    x = np.ascontiguousarray(np.asarray(x, dtype=np.float32))
    W_attn = np.ascontiguousarray(np.asarray(W_attn, dtype=np.float32))
    b_attn = np.ascontiguousarray(np.asarray(b_attn, dtype=np.float32))
    W_proj = np.ascontiguousarray(np.asarray(W_proj, dtype=np.float32))
    b_proj = np.ascontiguousarray(np.asarray(b_proj, dtype=np.float32))

    with_bias = bool(np.any(b_attn))
    nc = _get_nc(with_bias)

    bf = ml_dtypes.bfloat16

    def _ccp(w):
        # [768, n] -> [128, 6, n]: partition-major, contiguous per partition
        return np.ascontiguousarray(
            w.reshape(CC, 128, -1).transpose(1, 0, 2).astype(bf)
        )

    # x^T per batch as [qc, p, cc, t']: contiguous 6 KB per-partition lines
    xt_by_b = [
        np.ascontiguousarray(
            x[b].T.reshape(CC, 128, TC, 512).transpose(2, 1, 0, 3).astype(bf)
        )
        for b in range(B)
    ]
    tri = np.triu(np.ones((128, 128), dtype=np.float32)).astype(bf)
    trimask = np.ascontiguousarray(np.concatenate([tri, tri], axis=1))
    in_maps = []
    for c in range(8):
        b = c // 2
        hg = c % 2
        cs = slice(hg * 384, (hg + 1) * 384)
        wq = W_attn[:, 0:768][:, cs]
        wk = W_attn[:, 768:1536][:, cs]
        wvs = W_attn[:, 1536:2304][:, cs]
        wps = W_proj[cs, :]
        m = {
            "xt": xt_by_b[b],
            "msk": trimask,
            "wqk": _ccp(np.concatenate([wq, wk], axis=1)),
            "wv": _ccp(wvs),
            "wp": np.ascontiguousarray(
                wps.reshape(3, 128, 768).transpose(1, 0, 2).astype(bf)
            ),
        }
        if with_bias:
            m["bqk"] = np.ascontiguousarray(
                np.concatenate([b_attn[0:768][cs], b_attn[768:1536][cs]]).astype(bf)
            )[None, :]
            m["bv"] = np.ascontiguousarray(b_attn[1536:2304][cs].astype(bf))[None, :]
        in_maps.append(m)

    kwargs = _run_kwargs or {}
    res = run_bass_kernel_spmd(nc, in_maps, core_ids=list(range(8)), **kwargs)

    y = np.empty((B, T, C), dtype=np.float32)
    for b in range(B):
        y[b] = res.results[2 * b]["out"] + res.results[2 * b + 1]["out"]
    y += b_proj[None, None, :]
    if kwargs:
        kernel.last_result = res
    return y


# revision 62
# speedup vs baseline: 1.0073x; 1.0073x over previous
"""Causal self-attention (B=4, T=2048, C=768, 12 heads) on 8 trn2 NeuronCores.

Sharding: core c handles batch b = c//2 and head-group hg = c%2 (6 heads each).
Each core computes its 6 heads end-to-end plus its slice of the output
projection; the two head-group partial projections per batch are summed on the
host (one 6 MB add per batch).

Per-core pipeline (matmuls in bf16 with fp32 PSUM accumulation; softmax fp32):
  - x tiles cast to bf16 on host, transposed to xT; first chunk's DMA split
    per t-tile so the V matmuls start as soon as possible
  - qT/kT = W_qk^T x^T via PE (heads pair-packed into 128-partition tiles)
  - V in natural [t, hd] layout via PE with xT as the stationary operand,
    with a ones column appended per head for the softmax denominator
  - S^T[k,q] = K Q^T per head, two heads per PE slot via tile_position row
    packing (contraction dim = hd = 64); diagonal blocks compute only the
    live column range
  - exp on ACT straight out of PSUM (two heads per op) -> bf16 E; causal
    zero-fill via a host-built triangular bf16 mask multiplied on DVE
  - PV accumulation in PSUM over k tiles (M=65: 64 value dims + denominator)
  - normalization: den + raw-y staged out of PSUM immediately, den row DMA'd
    to partition 0, DVE reciprocal, GpSimd partition_broadcast, muls; the
    bcast+muls of each head pair are deferred one pair

Scheduling (v2): the exp load is strongly back-weighted (the last q-chunk has
5x the exp columns of the first) while its attention matmul work is tiny, so
phase-sequential emission starves either PE or ACT. Instead:
  - the A-phase (QKV) chains of chunk qc+1 and the projection of chunk qc-1
    are interleaved as PE filler UNITS between attention k-blocks (one eager
    filler at each head-pair boundary where S waits on the exp backlog);
  - A-phase PSUM->SBUF staging rides on ACT for early chunks (idle there)
    and on DVE for chunk 3 (ACT is exp-saturated by then);
  - DRAM inputs are host-prearranged partition-major so DMAs move multi-KB
    contiguous lines; wv/x chunk 0 are split so the first chains start early;
  - the final head pair is normalized per output QUARTER as each diagonal
    block's PV lands (quarter j is final after diag block j), reading raw y
    straight from PSUM, with the final projection t-tiles pipelined behind
    the quarter chains -- no keep-warm bridge needed at the tail.
"""

import ml_dtypes
import numpy as np

import concourse.bacc as bacc
import concourse.mybir as mybir
import concourse.tile as tile
from concourse.bass_utils import run_bass_kernel_spmd

F32 = mybir.dt.float32
BF16 = mybir.dt.bfloat16
AF = mybir.ActivationFunctionType

B, T, C = 4, 2048, 768
NH, HD = 12, 64
TC = 4  # t-chunks of 512
CC = 6  # contraction chunks of 128 over C
N_TT = 16  # t tiles of 128

_nc_cache = {}


def _build(with_bias: bool):
    nc = bacc.Bacc(None, target_bir_lowering=False)
    # host-prearranged layouts: partition dim first, contiguous per-partition
    # lines so the DRAM->SBUF DMAs move KB-sized packets
    xt_d = nc.dram_tensor("xt", [TC, 128, CC, 512], BF16, kind="ExternalInput")
    wqk = nc.dram_tensor("wqk", [128, CC, 768], BF16, kind="ExternalInput")
    wv = nc.dram_tensor("wv", [128, CC, 384], BF16, kind="ExternalInput")
    wp = nc.dram_tensor("wp", [128, 3, 768], BF16, kind="ExternalInput")
    msk = nc.dram_tensor("msk", [128, 256], BF16, kind="ExternalInput")
    if with_bias:
        bqk = nc.dram_tensor("bqk", [1, 768], BF16, kind="ExternalInput")
        bv = nc.dram_tensor("bv", [1, 384], BF16, kind="ExternalInput")
    out = nc.dram_tensor("out", [T, C], F32, kind="ExternalOutput")

    with tile.TileContext(nc) as tc, nc.allow_low_precision(
        reason="bf16 matmul operands are intentional"
    ):
        with (
            tc.tile_pool(name="const", bufs=1) as const,
            tc.tile_pool(name="xt", bufs=2) as xtp,
            tc.tile_pool(name="big", bufs=1) as big,
            tc.tile_pool(name="E", bufs=4) as epool,
            tc.tile_pool(name="norm", bufs=2) as npool,
            tc.tile_pool(name="ost", bufs=3) as opool,
            tc.tile_pool(name="ps", bufs=1, space="PSUM") as ps,
        ):
            # ---------------- constants ----------------
            # wdum memset first: it gates the PE warmup matmuls
            wdum = const.tile([128, 512], BF16, name="wdum")
            nc.vector.memset(wdum.bitcast(F32)[:, 0:256], 0.0)

            onecol_f = const.tile([128, 8], F32)
            nc.vector.memset(onecol_f, 1.0)
            onecol_bf = const.tile([128, 8], BF16)
            nc.vector.tensor_copy(onecol_bf, onecol_f)

            # warm the ACT exp table while DMAs run
            warm_f = const.tile([1, 8], F32)
            nc.scalar.activation(warm_f, onecol_f[0:1, :], AF.Exp, scale=1.0)

            if with_bias:
                ones_f = const.tile([1, 512], F32)
                nc.vector.memset(ones_f, 1.0)
                ones_bf = const.tile([1, 512], BF16)
                nc.vector.tensor_copy(ones_bf, ones_f)

            ones64_f = const.tile([1, 64], F32)
            nc.vector.memset(ones64_f, 1.0)

            # PE warmup: dummy matmuls with no input deps keep the HAM
            # activity monitor busy while the first DMAs land; long enough
            # that the clock gate opens (~3.4us) BEFORE the first real A(0)
            # chains, which otherwise run at half clock
            wps = ps.tile([128, 512], F32, tag="mm", bufs=2, name="warmps")
            for _ in range(14):
                nc.tensor.matmul(wps, wdum[:, 0:128], wdum, start=True, stop=True)
            warm_sb = const.tile([1, 8], F32, name="warmsb")
            nc.vector.tensor_copy(warm_sb, wps[0:1, 0:8])

            # ---------------- weights (bf16, host-prepared) ----------------
            # wv split per cc-pair so the first V chain starts after ~1/3
            wv_t = const.tile([128, CC, 384], BF16, name="wv_t")
            for ccp in range(3):
                nc.sync.dma_start(
                    out=wv_t[:, 2 * ccp : 2 * ccp + 2, :],
                    in_=wv[:, 2 * ccp : 2 * ccp + 2, :],
                )
            msk_sb = const.tile([128, 2, 128], BF16, name="msk_sb")
            wv_bf = [wv_t[:, cc, :] for cc in range(CC)]
            wqk_t = const.tile([128, CC, 768], BF16, name="wqk_t")
            wp_t = const.tile([128, 3, 768], BF16, name="wp_t")
            if with_bias:
                bqk_bf = const.tile([1, 768], BF16)
                bv_bf = const.tile([1, 384], BF16)

            # persistent big tiles
            qkT = [big.tile([128, T], BF16, name=f"qkT{ct}") for ct in range(6)]
            v_sb = [big.tile([128, 390], BF16, name=f"v{tt}") for tt in range(N_TT)]
            yT3 = [big.tile([128, T], BF16, name=f"yT{hp}") for hp in range(3)]

            # the denominator ones-columns of v_sb are constant: prefill once
            for tt in range(N_TT):
                vv = v_sb[tt].rearrange("p (h w) -> p h w", w=65)
                nc.vector.tensor_copy(vv[:, :, 64], onecol_bf[:, 0:6])

            def emit_xt_dma(qc):
                """Prefetch the x chunk for chunk qc; returns the tile."""
                xt_t = xtp.tile([128, CC, 512], BF16, tag="xt", name=f"xt_{qc}")
                if qc == 0:
                    # split the first chunk's DMA per contraction chunk so the
                    # first V/qk chains can start as soon as cc=0 lands
                    for cc in range(CC):
                        nc.sync.dma_start(
                            out=xt_t[:, cc, :], in_=xt_d[qc, :, cc, :]
                        )
                else:
                    nc.sync.dma_start(out=xt_t, in_=xt_d[qc, :, :, :])
                return xt_t

            def emit_v_unit(qc, xt_t, tt4):
                """V (+denominator ones) for one t-tile of chunk qc."""
                tt = qc * 4 + tt4
                xt_tiles = [xt_t[:, cc, :] for cc in range(CC)]
                v_ps = ps.tile([128, 384], F32, tag="mm", bufs=2, name=f"vps{tt}")
                for cc in range(CC):
                    nc.tensor.matmul(
                        v_ps,
                        xt_tiles[cc][:, tt4 * 128 : (tt4 + 1) * 128],
                        wv_bf[cc],
                        start=(cc == 0),
                        stop=(cc == CC - 1 and not with_bias),
                    )
                if with_bias:
                    nc.tensor.matmul(
                        v_ps, ones_bf[:, 0:128], bv_bf, start=False, stop=True
                    )
                vv = v_sb[tt].rearrange("p (h w) -> p h w", w=65)
                # early chunks' staging rides on ACT (idle then); chunk 3's
                # lands mid/late attention where ACT is exp-saturated -> DVE
                if qc >= 3:
                    nc.vector.tensor_copy(
                        vv[:, :, 0:64], v_ps.rearrange("p (h w) -> p h w", w=64)
                    )
                else:
                    nc.scalar.copy(
                        vv[:, :, 0:64], v_ps.rearrange("p (h w) -> p h w", w=64)
                    )

            def emit_qk_unit(qc, xt_t, ct):
                """One 128-col slice of qT/kT for chunk qc."""
                xt_tiles = [xt_t[:, cc, :] for cc in range(CC)]
                wqk_bf = [wqk_t[:, cc, :] for cc in range(CC)]
                qk_ps = ps.tile(
                    [128, 512], F32, tag="mm", bufs=2, name=f"qkps{qc}_{ct}"
                )
                for cc in range(CC):
                    nc.tensor.matmul(
                        qk_ps,
                        wqk_bf[cc][:, ct * 128 : (ct + 1) * 128],
                        xt_tiles[cc],
                        start=(cc == 0),
                        stop=(cc == CC - 1 and not with_bias),
                    )
                if with_bias:
                    nc.tensor.matmul(
                        qk_ps,
                        bqk_bf[:, ct * 128 : (ct + 1) * 128],
                        ones_bf,
                        start=False,
                        stop=True,
                    )
                if qc >= 3:
                    nc.vector.tensor_copy(
                        qkT[ct][:, qc * 512 : (qc + 1) * 512], qk_ps
                    )
                else:
                    nc.scalar.copy(qkT[ct][:, qc * 512 : (qc + 1) * 512], qk_ps)

            def emit_proj_unit(qc, tt):
                """Output projection + store for one t-tile of chunk qc."""
                t_sl = slice(tt * 128, (tt + 1) * 128)
                ostage = opool.tile([128, 768], F32, tag="ost")
                for half in range(2):
                    pp = ps.tile(
                        [128, 384], F32, tag="mm", bufs=2, name=f"pj{tt}_{half}"
                    )
                    for hp in range(3):
                        nc.tensor.matmul(
                            pp,
                            yT3[hp][:, t_sl],
                            wp_t[:, hp, half * 384 : (half + 1) * 384],
                            start=(hp == 0),
                            stop=(hp == 2),
                        )
                    nc.vector.tensor_copy(
                        ostage[:, half * 384 : (half + 1) * 384], pp
                    )
                nc.sync.dma_start(out=out[t_sl, :], in_=ostage)

            def emit_attention(qc, fillers):
                """Attention for chunk qc with PE filler units interleaved
                between k-blocks. Returns (last_yst, pending_finishes)."""
                q_sl = slice(qc * 512, (qc + 1) * 512)
                n_kt = 4 * qc + 4
                nblocks = 3 * n_kt
                nfill = len(fillers)
                bi = 0
                fi = 0

                def tick():
                    # advance the filler schedule after each k-block
                    nonlocal bi, fi
                    bi += 1
                    while fi < nfill and fi * nblocks < bi * nfill:
                        fillers[fi]()
                        fi += 1

                tail_quarters = []

                def _tail_quarter_p1(j, yT_a, yT_b):
                    """Phase 1 of the final head pair's quarter-j norm chain
                    (den row staging + DMA to partition 0). Emitted right
                    after diag block j's PV: only cheap wait-free DVE ops, so
                    the remaining attention blocks are not stalled. Phase 2
                    (DMA-dependent) is deferred past the kt loop."""
                    qsl_j = slice(j * 128, (j + 1) * 128)
                    denq = npool.tile([128, 256], F32, tag="denq", bufs=4)
                    nc.vector.tensor_copy(denq[64:65, 0:128], yT_a[64:65, qsl_j])
                    nc.vector.tensor_copy(denq[64:65, 128:256], yT_b[64:65, qsl_j])
                    d0q = npool.tile([1, 512], F32, tag="d0q", bufs=4)
                    nc.sync.dma_start(out=d0q[0:1, 0:256], in_=denq[64:65, :])
                    tail_quarters.append((j, d0q, yT_a, yT_b))

                def _tail_quarter_p2(j, d0q, yT_a, yT_b):
                    """Phase 2: reciprocal, broadcast, PSUM-direct muls (all
                    base-0 operands), partition-shift DMA. The projection
                    t-tiles are emitted separately so the PE queue order
                    keeps all attention matmuls first."""
                    qsl_j = slice(j * 128, (j + 1) * 128)
                    q_abs = slice(qc * 512 + j * 128, qc * 512 + (j + 1) * 128)
                    nc.vector.reciprocal_approx_fast(
                        d0q[0:1, 256:512], d0q[0:1, 0:256]
                    )
                    recq = npool.tile([64, 256], F32, tag="recq", bufs=4)
                    nc.gpsimd.partition_broadcast(
                        recq, d0q[0:1, 256:512], channels=64
                    )
                    nc.vector.tensor_mul(
                        yT3[2][0:64, q_abs], yT_a[0:64, qsl_j], recq[:, 0:128]
                    )
                    ytq = npool.tile([64, 128], BF16, tag="ytq", bufs=4)
                    nc.vector.tensor_mul(ytq, yT_b[0:64, qsl_j], recq[:, 128:256])
                    nc.sync.dma_start(out=yT3[2][64:128, q_abs], in_=ytq)

                pending = []
                last_den = None
                for hp in range(3):
                    if fi < nfill:
                        # one eager filler at each head-pair boundary: the new
                        # pair's first S waits on the previous pair's exp
                        # backlog, so give PE something to chew on
                        fillers[fi]()
                        fi += 1
                    yT_a = ps.tile([65, 512], F32, tag="yT", bufs=2, name=f"ya{qc}_{hp}")
                    yT_b = ps.tile([65, 512], F32, tag="yT", bufs=2, name=f"yb{qc}_{hp}")
                    e_hist = []
                    for kt in range(n_kt):
                        k_sl = slice(kt * 128, (kt + 1) * 128)
                        m = kt - 4 * qc
                        diag = m >= 0
                        w = 512 - 128 * max(m, 0)  # live column range
                        f0 = 512 - w
                        psS = ps.tile(
                            [128, 1024], F32, tag="S", bufs=2, name=f"s{qc}_{hp}_{kt}"
                        )
                        nc.tensor.matmul(
                            psS[:, f0:512],
                            qkT[3 + hp][0:64, k_sl],
                            qkT[hp][0:64, qc * 512 + f0 : (qc + 1) * 512],
                            start=True,
                            stop=True,
                            tile_position=(0, 0),
                        )
                        nc.tensor.matmul(
                            psS[:, 512 + f0 : 1024],
                            qkT[3 + hp][64:128, k_sl],
                            qkT[hp][64:128, qc * 512 + f0 : (qc + 1) * 512],
                            start=True,
                            stop=True,
                            tile_position=(64, 0),
                        )
                        E = epool.tile(
                            [128, 1024], BF16, tag="E", bufs=8, name=f"e{qc}_{hp}_{kt}"
                        )
                        psv = psS.rearrange("p (h w) -> p h w", w=512)
                        ev = E.rearrange("p (h w) -> p h w", w=512)
                        nc.scalar.activation(
                            ev[:, :, f0:512], psv[:, :, f0:512], AF.Exp, scale=0.125
                        )
                        if diag:
                            # causal zero-fill via host-built triangular mask
                            # (DVE: GpSimd's dispatch latency stalls the
                            # E->PV chain even though it has idle capacity)
                            nc.vector.tensor_mul(
                                ev[:, :, f0 : f0 + 128],
                                ev[:, :, f0 : f0 + 128],
                                msk_sb,
                            )
                        e_hist.append((kt, E, f0))
                        tail_pair = qc == TC - 1 and hp == 2
                        if tail_pair and kt >= 4 * qc:
                            # tail head pair, diagonal region: drain PV
                            # eagerly; quarter j of the output columns is
                            # final once diag block m=j has accumulated, so
                            # its normalization chain and its projection
                            # t-tile can start while attention still runs
                            while e_hist:
                                _pv(nc, v_sb, yT_a, yT_b, hp, *e_hist.pop(0), n_kt)
                            if pending:
                                pending.pop(0)()
                            _tail_quarter_p1(kt - 4 * qc, yT_a, yT_b)
                            _tail_quarter_p2(*tail_quarters.pop(0))
                            emit_proj_unit(qc, kt)
                        elif len(e_hist) > 3:
                            # drain two blocks at once: fewer S<->PV
                            # transitions means fewer exposed PV weight loads
                            _pv(nc, v_sb, yT_a, yT_b, hp, *e_hist.pop(0), n_kt)
                            _pv(nc, v_sb, yT_a, yT_b, hp, *e_hist.pop(0), n_kt)
                        tick()
                    if qc == TC - 1 and hp == 2:
                        continue  # tail handled per-quarter above
                    while e_hist:
                        _pv(nc, v_sb, yT_a, yT_b, hp, *e_hist.pop(0), n_kt)

                    # stage denominator + raw y out of PSUM immediately (frees
                    # the yT banks); den row DMA'd to partition 0 (DVE recip
                    # ucode requires base 0). The broadcast + muls are
                    # deferred one head pair.
                    den = npool.tile(
                        [128, 1024], F32, tag="den", bufs=2, name=f"dn{qc}_{hp}"
                    )
                    nc.vector.tensor_copy(den[64:65, 0:512], yT_a[64:65, :])
                    nc.vector.tensor_copy(den[64:65, 512:1024], yT_b[64:65, :])
                    last_den = den
                    yst = npool.tile(
                        [64, 1024], BF16, tag="yst", bufs=3, name=f"ys{qc}_{hp}"
                    )
                    nc.vector.tensor_copy(yst[:, 0:512], yT_a[0:64, :])
                    nc.vector.tensor_copy(yst[:, 512:1024], yT_b[0:64, :])
                    den0 = npool.tile(
                        [1, 2048], F32, tag="den0", bufs=3, name=f"d0{qc}_{hp}"
                    )
                    nc.sync.dma_start(out=den0[0:1, 0:1024], in_=den[64:65, :])

                    def finish(hp=hp, yst=yst, den0=den0):
                        nc.vector.reciprocal_approx_fast(
                            den0[0:1, 1024:2048], den0[0:1, 0:1024]
                        )
                        rec = npool.tile(
                            [64, 1024], F32, tag="rec", bufs=3, name=f"rc{qc}_{hp}"
                        )
                        nc.gpsimd.partition_broadcast(
                            rec, den0[0:1, 1024:2048], channels=64
                        )
                        nc.vector.tensor_mul(
                            yT3[hp][0:64, q_sl], yst[:, 0:512], rec[:, 0:512]
                        )
                        ytmp = npool.tile(
                            [64, 512], BF16, tag="ytmp", bufs=2, name=f"yt{qc}_{hp}"
                        )
                        nc.vector.tensor_mul(ytmp, yst[:, 512:1024], rec[:, 512:1024])
                        # partition shift 0:64 -> 64:128 via SBUF->SBUF DMA
                        nc.sync.dma_start(out=yT3[hp][64:128, q_sl], in_=ytmp)

                    if pending:
                        pending.pop(0)()
                    pending.append(finish)
                while fi < nfill:
                    fillers[fi]()
                    fi += 1
                return last_den, pending

            # ---------------- main schedule ----------------
            # A(0) emitted directly (nothing to interleave with); for qc>=0
            # attention(qc) interleaves the A-chains of qc+1 and the
            # projection of qc-1 as PE filler units.
            xt0 = emit_xt_dma(0)
            nc.sync.dma_start(out=wqk_t[:, 0:3, :], in_=wqk[:, 0:3, :])
            nc.sync.dma_start(out=wqk_t[:, 3:6, :], in_=wqk[:, 3:6, :])
            nc.sync.dma_start(
                out=msk_sb, in_=msk.rearrange("p (h w) -> p h w", w=128)
            )
            if with_bias:
                nc.sync.dma_start(out=bqk_bf, in_=bqk[:, :])
                nc.sync.dma_start(out=bv_bf, in_=bv[:, :])
            for tt4 in range(4):
                emit_v_unit(0, xt0, tt4)
            for ct in range(6):
                emit_qk_unit(0, xt0, ct)
            nc.sync.dma_start(out=wp_t, in_=wp[:, :, :])

            pend_prev = []
            xt_next = None
            for qc in range(TC):
                if qc < TC - 1:
                    xt_next = emit_xt_dma(qc + 1)
                    xt_cap = xt_next

                    def mk_v(tt4, xt_cap=xt_cap, q1=qc + 1):
                        return lambda: emit_v_unit(q1, xt_cap, tt4)

                    def mk_qk(ct, xt_cap=xt_cap, q1=qc + 1):
                        return lambda: emit_qk_unit(q1, xt_cap, ct)

                    aunits = [mk_v(tt4) for tt4 in range(4)] + [
                        mk_qk(ct) for ct in range(6)
                    ]
                else:
                    aunits = []
                if qc >= 1:
                    punits = [
                        (lambda tt=tt: emit_proj_unit(qc - 1, tt))
                        for tt in range((qc - 1) * 4, (qc - 1) * 4 + 4)
                    ]
                else:
                    punits = []
                # filler order: two A-units, then the previous chunk's
                # deferred norm finish (its den0 DMA needs latency headroom,
                # but it must precede the proj units), then alternate the
                # remaining A-units with the proj units
                fillers = aunits[:2] + list(pend_prev)
                rest_a = aunits[2:]
                ai, pi = 0, 0
                for i in range(len(rest_a) + len(punits)):
                    if ai < len(rest_a) and (pi >= len(punits) or i % 2 == 0):
                        fillers.append(rest_a[ai])
                        ai += 1
                    else:
                        fillers.append(punits[pi])
                        pi += 1
                last_den, pend_prev = emit_attention(qc, fillers)
            # tail norm chains + final projection are emitted inside the last
            # chunk's attention (quarter-pipelined per diagonal block)

    nc.finalize()
    return nc


def _pv(nc, v_sb, yT_a, yT_b, hp, kt, E, f0, n_kt):
    a = 2 * hp
    nc.tensor.matmul(
        yT_a[:, f0:512],
        v_sb[kt][:, a * 65 : (a + 1) * 65],
        E[:, f0:512],
        start=(kt == 0),
        stop=(kt == n_kt - 1),
    )
    nc.tensor.matmul(
        yT_b[:, f0:512],
        v_sb[kt][:, (a + 1) * 65 : (a + 2) * 65],
        E[:, 512 + f0 : 1024],
        start=(kt == 0),
        stop=(kt == n_kt - 1),
    )


def _get_nc(with_bias: bool):
    if with_bias not in _nc_cache:
        _nc_cache[with_bias] = _build(with_bias)
    return _nc_cache[with_bias]


def kernel(x, W_attn, b_attn, W_proj, b_proj, _run_kwargs=None):
    x = np.ascontiguousarray(np.asarray(x, dtype=np.float32))
    W_attn = np.ascontiguousarray(np.asarray(W_attn, dtype=np.float32))
    b_attn = np.ascontiguousarray(np.asarray(b_attn, dtype=np.float32))
    W_proj = np.ascontiguousarray(np.asarray(W_proj, dtype=np.float32))
    b_proj = np.ascontiguousarray(np.asarray(b_proj, dtype=np.float32))

    with_bias = bool(np.any(b_attn))
    nc = _get_nc(with_bias)

    bf = ml_dtypes.bfloat16

    def _ccp(w):
        # [768, n] -> [128, 6, n]: partition-major, contiguous per partition
        return np.ascontiguousarray(
            w.reshape(CC, 128, -1).transpose(1, 0, 2).astype(bf)
        )

    # x^T per batch as [qc, p, cc, t']: contiguous 6 KB per-partition lines
    xt_by_b = [
        np.ascontiguousarray(
            x[b].T.reshape(CC, 128, TC, 512).transpose(2, 1, 0, 3).astype(bf)
        )
        for b in range(B)
    ]
    tri = np.triu(np.ones((128, 128), dtype=np.float32)).astype(bf)
    trimask = np.ascontiguousarray(np.concatenate([tri, tri], axis=1))
    in_maps = []
    for c in range(8):
        b = c // 2
        hg = c % 2
        cs = slice(hg * 384, (hg + 1) * 384)
        wq = W_attn[:, 0:768][:, cs]
        wk = W_attn[:, 768:1536][:, cs]
        wvs = W_attn[:, 1536:2304][:, cs]
        wps = W_proj[cs, :]
        m = {
            "xt": xt_by_b[b],
            "msk": trimask,
            "wqk": _ccp(np.concatenate([wq, wk], axis=1)),
            "wv": _ccp(wvs),
            "wp": np.ascontiguousarray(
                wps.reshape(3, 128, 768).transpose(1, 0, 2).astype(bf)
            ),
        }
        if with_bias:
            m["bqk"] = np.ascontiguousarray(
                np.concatenate([b_attn[0:768][cs], b_attn[768:1536][cs]]).astype(bf)
            )[None, :]
            m["bv"] = np.ascontiguousarray(b_attn[1536:2304][cs].astype(bf))[None, :]
        in_maps.append(m)

    kwargs = _run_kwargs or {}
    res = run_bass_kernel_spmd(nc, in_maps, core_ids=list(range(8)), **kwargs)

    y = np.empty((B, T, C), dtype=np.float32)
    for b in range(B):
        y[b] = res.results[2 * b]["out"] + res.results[2 * b + 1]["out"]
    y += b_proj[None, None, :]
    if kwargs:
        kernel.last_result = res
    return y


# revision 64
# speedup vs baseline: 1.0216x; 1.0142x over previous
"""Causal self-attention (B=4, T=2048, C=768, 12 heads) on 8 trn2 NeuronCores.

Sharding: core c handles batch b = c//2 and head-group hg = c%2 (6 heads each).
Each core computes its 6 heads end-to-end plus its slice of the output
projection; the two head-group partial projections per batch are summed on the
host (one 6 MB add per batch).

Per-core pipeline (matmuls in bf16 with fp32 PSUM accumulation; softmax fp32):
  - x tiles cast to bf16 on host, transposed to xT; first chunk's DMA split
    per t-tile so the V matmuls start as soon as possible
  - qT/kT = W_qk^T x^T via PE (heads pair-packed into 128-partition tiles)
  - V in natural [t, hd] layout via PE with xT as the stationary operand,
    with a ones column appended per head for the softmax denominator
  - S^T[k,q] = K Q^T per head, two heads per PE slot via tile_position row
    packing (contraction dim = hd = 64); diagonal blocks compute only the
    live column range
  - exp on ACT straight out of PSUM (two heads per op) -> bf16 E; causal
    zero-fill via a host-built triangular bf16 mask multiplied on DVE
  - PV accumulation in PSUM over k tiles (M=65: 64 value dims + denominator)
  - normalization: den + raw-y staged out of PSUM immediately, den row DMA'd
    to partition 0, DVE reciprocal, GpSimd partition_broadcast, muls; the
    bcast+muls of each head pair are deferred one pair

Scheduling (v2): the exp load is strongly back-weighted (the last q-chunk has
5x the exp columns of the first) while its attention matmul work is tiny, so
phase-sequential emission starves either PE or ACT. Instead:
  - the A-phase (QKV) chains of chunk qc+1 and the projection of chunk qc-1
    are interleaved as PE filler UNITS between attention k-blocks (one eager
    filler at each head-pair boundary where S waits on the exp backlog);
  - A-phase PSUM->SBUF staging rides on ACT for early chunks (idle there)
    and on DVE for chunk 3 (ACT is exp-saturated by then);
  - DRAM inputs are host-prearranged partition-major so DMAs move multi-KB
    contiguous lines; wv/x chunk 0 are split so the first chains start early;
  - the final head pair is normalized per output QUARTER as each diagonal
    block's PV lands (quarter j is final after diag block j), reading raw y
    straight from PSUM, with the final projection t-tiles pipelined behind
    the quarter chains -- no keep-warm bridge needed at the tail.
"""

import ml_dtypes
import numpy as np

import concourse.bacc as bacc
import concourse.mybir as mybir
import concourse.tile as tile
from concourse.bass_utils import run_bass_kernel_spmd

F32 = mybir.dt.float32
BF16 = mybir.dt.bfloat16
AF = mybir.ActivationFunctionType

B, T, C = 4, 2048, 768
NH, HD = 12, 64
TC = 4  # t-chunks of 512
CC = 6  # contraction chunks of 128 over C
N_TT = 16  # t tiles of 128

_nc_cache = {}


def _build(with_bias: bool):
    nc = bacc.Bacc(None, target_bir_lowering=False)
    # host-prearranged layouts: partition dim first, contiguous per-partition
    # lines so the DRAM->SBUF DMAs move KB-sized packets
    xt_d = nc.dram_tensor("xt", [TC, 128, CC, 512], BF16, kind="ExternalInput")
    wqk = nc.dram_tensor("wqk", [128, CC, 768], BF16, kind="ExternalInput")
    wv = nc.dram_tensor("wv", [128, CC, 384], BF16, kind="ExternalInput")
    wp = nc.dram_tensor("wp", [128, 3, 768], BF16, kind="ExternalInput")
    msk = nc.dram_tensor("msk", [128, 256], BF16, kind="ExternalInput")
    if with_bias:
        bqk = nc.dram_tensor("bqk", [1, 768], BF16, kind="ExternalInput")
        bv = nc.dram_tensor("bv", [1, 384], BF16, kind="ExternalInput")
    out = nc.dram_tensor("out", [T, C], F32, kind="ExternalOutput")

    with tile.TileContext(nc) as tc, nc.allow_low_precision(
        reason="bf16 matmul operands are intentional"
    ):
        with (
            tc.tile_pool(name="const", bufs=1) as const,
            tc.tile_pool(name="xt", bufs=2) as xtp,
            tc.tile_pool(name="big", bufs=1) as big,
            tc.tile_pool(name="E", bufs=4) as epool,
            tc.tile_pool(name="norm", bufs=2) as npool,
            tc.tile_pool(name="ost", bufs=3) as opool,
            tc.tile_pool(name="ps", bufs=1, space="PSUM") as ps,
        ):
            # ---------------- constants ----------------
            # wdum memset first: it gates the PE warmup matmuls
            wdum = const.tile([128, 512], BF16, name="wdum")
            nc.vector.memset(wdum.bitcast(F32)[:, 0:256], 0.0)

            onecol_f = const.tile([128, 8], F32)
            nc.vector.memset(onecol_f, 1.0)
            onecol_bf = const.tile([128, 8], BF16)
            nc.vector.tensor_copy(onecol_bf, onecol_f)

            # warm the ACT exp table while DMAs run
            warm_f = const.tile([1, 8], F32)
            nc.scalar.activation(warm_f, onecol_f[0:1, :], AF.Exp, scale=1.0)

            if with_bias:
                ones_f = const.tile([1, 512], F32)
                nc.vector.memset(ones_f, 1.0)
                ones_bf = const.tile([1, 512], BF16)
                nc.vector.tensor_copy(ones_bf, ones_f)

            ones64_f = const.tile([1, 64], F32)
            nc.vector.memset(ones64_f, 1.0)

            # PE warmup: dummy matmuls with no input deps keep the HAM
            # activity monitor busy while the first DMAs land; long enough
            # that the clock gate opens (~3.4us) BEFORE the first real A(0)
            # chains, which otherwise run at half clock
            wps = ps.tile([128, 512], F32, tag="mm", bufs=2, name="warmps")
            for _ in range(14):
                nc.tensor.matmul(wps, wdum[:, 0:128], wdum, start=True, stop=True)
            warm_sb = const.tile([1, 8], F32, name="warmsb")
            nc.vector.tensor_copy(warm_sb, wps[0:1, 0:8])

            # ---------------- weights (bf16, host-prepared) ----------------
            # wv split per cc-pair so the first V chain starts after ~1/3
            wv_t = const.tile([128, CC, 384], BF16, name="wv_t")
            for ccp in range(3):
                nc.sync.dma_start(
                    out=wv_t[:, 2 * ccp : 2 * ccp + 2, :],
                    in_=wv[:, 2 * ccp : 2 * ccp + 2, :],
                )
            msk_sb = const.tile([128, 2, 128], BF16, name="msk_sb")
            wv_bf = [wv_t[:, cc, :] for cc in range(CC)]
            wqk_t = const.tile([128, CC, 768], BF16, name="wqk_t")
            wp_t = const.tile([128, 3, 768], BF16, name="wp_t")
            if with_bias:
                bqk_bf = const.tile([1, 768], BF16)
                bv_bf = const.tile([1, 384], BF16)

            # persistent big tiles
            qkT = [big.tile([128, T], BF16, name=f"qkT{ct}") for ct in range(6)]
            v_sb = [big.tile([128, 390], BF16, name=f"v{tt}") for tt in range(N_TT)]
            yT3 = [big.tile([128, T], BF16, name=f"yT{hp}") for hp in range(3)]

            # the denominator ones-columns of v_sb are constant: prefill once
            for tt in range(N_TT):
                vv = v_sb[tt].rearrange("p (h w) -> p h w", w=65)
                nc.vector.tensor_copy(vv[:, :, 64], onecol_bf[:, 0:6])

            def emit_xt_dma(qc):
                """Prefetch the x chunk for chunk qc; returns the tile."""
                xt_t = xtp.tile([128, CC, 512], BF16, tag="xt", name=f"xt_{qc}")
                if qc == 0:
                    # split the first chunk's DMA per contraction chunk so the
                    # first V/qk chains can start as soon as cc=0 lands
                    for cc in range(CC):
                        nc.sync.dma_start(
                            out=xt_t[:, cc, :], in_=xt_d[qc, :, cc, :]
                        )
                else:
                    nc.sync.dma_start(out=xt_t, in_=xt_d[qc, :, :, :])
                return xt_t

            def emit_v_unit(qc, xt_t, tt4):
                """V (+denominator ones) for one t-tile of chunk qc."""
                tt = qc * 4 + tt4
                xt_tiles = [xt_t[:, cc, :] for cc in range(CC)]
                v_ps = ps.tile([128, 384], F32, tag="mm", bufs=2, name=f"vps{tt}")
                for cc in range(CC):
                    nc.tensor.matmul(
                        v_ps,
                        xt_tiles[cc][:, tt4 * 128 : (tt4 + 1) * 128],
                        wv_bf[cc],
                        start=(cc == 0),
                        stop=(cc == CC - 1 and not with_bias),
                    )
                if with_bias:
                    nc.tensor.matmul(
                        v_ps, ones_bf[:, 0:128], bv_bf, start=False, stop=True
                    )
                vv = v_sb[tt].rearrange("p (h w) -> p h w", w=65)
                # early chunks' staging rides on ACT (idle then); chunk 3's
                # lands mid/late attention where ACT is exp-saturated -> DVE
                if qc >= 3:
                    nc.vector.tensor_copy(
                        vv[:, :, 0:64], v_ps.rearrange("p (h w) -> p h w", w=64)
                    )
                else:
                    nc.scalar.copy(
                        vv[:, :, 0:64], v_ps.rearrange("p (h w) -> p h w", w=64)
                    )

            def emit_qk_unit(qc, xt_t, ct):
                """One 128-col slice of qT/kT for chunk qc."""
                xt_tiles = [xt_t[:, cc, :] for cc in range(CC)]
                wqk_bf = [wqk_t[:, cc, :] for cc in range(CC)]
                qk_ps = ps.tile(
                    [128, 512], F32, tag="mm", bufs=2, name=f"qkps{qc}_{ct}"
                )
                for cc in range(CC):
                    nc.tensor.matmul(
                        qk_ps,
                        wqk_bf[cc][:, ct * 128 : (ct + 1) * 128],
                        xt_tiles[cc],
                        start=(cc == 0),
                        stop=(cc == CC - 1 and not with_bias),
                    )
                if with_bias:
                    nc.tensor.matmul(
                        qk_ps,
                        bqk_bf[:, ct * 128 : (ct + 1) * 128],
                        ones_bf,
                        start=False,
                        stop=True,
                    )
                if qc >= 3:
                    nc.vector.tensor_copy(
                        qkT[ct][:, qc * 512 : (qc + 1) * 512], qk_ps
                    )
                else:
                    nc.scalar.copy(qkT[ct][:, qc * 512 : (qc + 1) * 512], qk_ps)

            def emit_proj_unit(qc, tt):
                """Output projection + store for one t-tile of chunk qc."""
                t_sl = slice(tt * 128, (tt + 1) * 128)
                ostage = opool.tile([128, 768], F32, tag="ost")
                for half in range(2):
                    pp = ps.tile(
                        [128, 384], F32, tag="mm", bufs=2, name=f"pj{tt}_{half}"
                    )
                    for hp in range(3):
                        nc.tensor.matmul(
                            pp,
                            yT3[hp][:, t_sl],
                            wp_t[:, hp, half * 384 : (half + 1) * 384],
                            start=(hp == 0),
                            stop=(hp == 2),
                        )
                    nc.vector.tensor_copy(
                        ostage[:, half * 384 : (half + 1) * 384], pp
                    )
                nc.sync.dma_start(out=out[t_sl, :], in_=ostage)

            def emit_attention(qc, fillers):
                """Attention for chunk qc with PE filler units interleaved
                between k-blocks. Returns (last_yst, pending_finishes)."""
                q_sl = slice(qc * 512, (qc + 1) * 512)
                n_kt = 4 * qc + 4
                nblocks = 3 * n_kt
                nfill = len(fillers)
                bi = 0
                fi = 0

                def tick():
                    # advance the filler schedule after each k-block
                    nonlocal bi, fi
                    bi += 1
                    while fi < nfill and fi * nblocks < bi * nfill:
                        fillers[fi]()
                        fi += 1

                tail_quarters = []

                def _tail_quarter_p1(j, yT_a, yT_b):
                    """Phase 1 of the final head pair's quarter-j norm chain
                    (den row staging + DMA to partition 0). Emitted right
                    after diag block j's PV: only cheap wait-free DVE ops, so
                    the remaining attention blocks are not stalled. Phase 2
                    (DMA-dependent) is deferred past the kt loop."""
                    qsl_j = slice(j * 128, (j + 1) * 128)
                    denq = npool.tile([128, 256], F32, tag="denq", bufs=4)
                    nc.vector.tensor_copy(denq[64:65, 0:128], yT_a[64:65, qsl_j])
                    nc.vector.tensor_copy(denq[64:65, 128:256], yT_b[64:65, qsl_j])
                    d0q = npool.tile([1, 512], F32, tag="d0q", bufs=4)
                    nc.sync.dma_start(out=d0q[0:1, 0:256], in_=denq[64:65, :])
                    tail_quarters.append((j, d0q, yT_a, yT_b))

                def _tail_quarter_p2(j, d0q, yT_a, yT_b):
                    """Phase 2: reciprocal, broadcast, PSUM-direct muls (all
                    base-0 operands), partition-shift DMA. The projection
                    t-tiles are emitted separately so the PE queue order
                    keeps all attention matmuls first."""
                    qsl_j = slice(j * 128, (j + 1) * 128)
                    q_abs = slice(qc * 512 + j * 128, qc * 512 + (j + 1) * 128)
                    nc.vector.reciprocal_approx_fast(
                        d0q[0:1, 256:512], d0q[0:1, 0:256]
                    )
                    recq = npool.tile([64, 256], F32, tag="recq", bufs=4)
                    nc.gpsimd.partition_broadcast(
                        recq, d0q[0:1, 256:512], channels=64
                    )
                    nc.vector.tensor_mul(
                        yT3[2][0:64, q_abs], yT_a[0:64, qsl_j], recq[:, 0:128]
                    )
                    ytq = npool.tile([64, 128], BF16, tag="ytq", bufs=4)
                    nc.vector.tensor_mul(ytq, yT_b[0:64, qsl_j], recq[:, 128:256])
                    nc.sync.dma_start(out=yT3[2][64:128, q_abs], in_=ytq)

                pending = []
                last_den = None
                for hp in range(3):
                    if fi < nfill:
                        # one eager filler at each head-pair boundary: the new
                        # pair's first S waits on the previous pair's exp
                        # backlog, so give PE something to chew on
                        fillers[fi]()
                        fi += 1
                    yT_a = ps.tile([65, 512], F32, tag="yT", bufs=2, name=f"ya{qc}_{hp}")
                    yT_b = ps.tile([65, 512], F32, tag="yT", bufs=2, name=f"yb{qc}_{hp}")
                    e_hist = []
                    for kt in range(n_kt):
                        k_sl = slice(kt * 128, (kt + 1) * 128)
                        m = kt - 4 * qc
                        diag = m >= 0
                        w = 512 - 128 * max(m, 0)  # live column range
                        f0 = 512 - w
                        psS = ps.tile(
                            [128, 1024], F32, tag="S", bufs=2, name=f"s{qc}_{hp}_{kt}"
                        )
                        nc.tensor.matmul(
                            psS[:, f0:512],
                            qkT[3 + hp][0:64, k_sl],
                            qkT[hp][0:64, qc * 512 + f0 : (qc + 1) * 512],
                            start=True,
                            stop=True,
                            tile_position=(0, 0),
                        )
                        nc.tensor.matmul(
                            psS[:, 512 + f0 : 1024],
                            qkT[3 + hp][64:128, k_sl],
                            qkT[hp][64:128, qc * 512 + f0 : (qc + 1) * 512],
                            start=True,
                            stop=True,
                            tile_position=(64, 0),
                        )
                        E = epool.tile(
                            [128, 1024], BF16, tag="E", bufs=8, name=f"e{qc}_{hp}_{kt}"
                        )
                        psv = psS.rearrange("p (h w) -> p h w", w=512)
                        ev = E.rearrange("p (h w) -> p h w", w=512)
                        nc.scalar.activation(
                            ev[:, :, f0:512], psv[:, :, f0:512], AF.Exp, scale=0.125
                        )
                        if diag:
                            # causal zero-fill via host-built triangular mask
                            # (DVE: GpSimd's dispatch latency stalls the
                            # E->PV chain even though it has idle capacity)
                            nc.vector.tensor_mul(
                                ev[:, :, f0 : f0 + 128],
                                ev[:, :, f0 : f0 + 128],
                                msk_sb,
                            )
                        e_hist.append((kt, E, f0))
                        tail_pair = qc == TC - 1 and hp == 2
                        if tail_pair and kt >= 4 * qc:
                            # tail head pair, diagonal region: drain PV
                            # eagerly; quarter j of the output columns is
                            # final once diag block m=j has accumulated, so
                            # its normalization chain and its projection
                            # t-tile can start while attention still runs
                            while e_hist:
                                _pv(nc, v_sb, yT_a, yT_b, hp, *e_hist.pop(0), n_kt)
                            if pending:
                                pending.pop(0)()
                            _tail_quarter_p1(kt - 4 * qc, yT_a, yT_b)
                        elif len(e_hist) > 3:
                            # drain two blocks at once: fewer S<->PV
                            # transitions means fewer exposed PV weight loads
                            _pv(nc, v_sb, yT_a, yT_b, hp, *e_hist.pop(0), n_kt)
                            _pv(nc, v_sb, yT_a, yT_b, hp, *e_hist.pop(0), n_kt)
                        tick()
                    if qc == TC - 1 and hp == 2:
                        # all attention matmuls are now emitted; run the
                        # quarter chains + projections behind them so the PE
                        # queue is never blocked mid-attention
                        for j in range(4):
                            _tail_quarter_p2(*tail_quarters.pop(0))
                            emit_proj_unit(qc, 4 * qc + j)
                        continue
                    while e_hist:
                        _pv(nc, v_sb, yT_a, yT_b, hp, *e_hist.pop(0), n_kt)

                    # stage denominator + raw y out of PSUM immediately (frees
                    # the yT banks); den row DMA'd to partition 0 (DVE recip
                    # ucode requires base 0). The broadcast + muls are
                    # deferred one head pair.
                    den = npool.tile(
                        [128, 1024], F32, tag="den", bufs=2, name=f"dn{qc}_{hp}"
                    )
                    nc.vector.tensor_copy(den[64:65, 0:512], yT_a[64:65, :])
                    nc.vector.tensor_copy(den[64:65, 512:1024], yT_b[64:65, :])
                    last_den = den
                    yst = npool.tile(
                        [64, 1024], BF16, tag="yst", bufs=3, name=f"ys{qc}_{hp}"
                    )
                    nc.vector.tensor_copy(yst[:, 0:512], yT_a[0:64, :])
                    nc.vector.tensor_copy(yst[:, 512:1024], yT_b[0:64, :])
                    den0 = npool.tile(
                        [1, 2048], F32, tag="den0", bufs=3, name=f"d0{qc}_{hp}"
                    )
                    nc.sync.dma_start(out=den0[0:1, 0:1024], in_=den[64:65, :])

                    def finish(hp=hp, yst=yst, den0=den0):
                        nc.vector.reciprocal_approx_fast(
                            den0[0:1, 1024:2048], den0[0:1, 0:1024]
                        )
                        rec = npool.tile(
                            [64, 1024], F32, tag="rec", bufs=3, name=f"rc{qc}_{hp}"
                        )
                        nc.gpsimd.partition_broadcast(
                            rec, den0[0:1, 1024:2048], channels=64
                        )
                        nc.vector.tensor_mul(
                            yT3[hp][0:64, q_sl], yst[:, 0:512], rec[:, 0:512]
                        )
                        ytmp = npool.tile(
                            [64, 512], BF16, tag="ytmp", bufs=2, name=f"yt{qc}_{hp}"
                        )
                        nc.vector.tensor_mul(ytmp, yst[:, 512:1024], rec[:, 512:1024])
                        # partition shift 0:64 -> 64:128 via SBUF->SBUF DMA
                        nc.sync.dma_start(out=yT3[hp][64:128, q_sl], in_=ytmp)

                    if pending:
                        pending.pop(0)()
                    pending.append(finish)
                while fi < nfill:
                    fillers[fi]()
                    fi += 1
                return last_den, pending

            # ---------------- main schedule ----------------
            # A(0) emitted directly (nothing to interleave with); for qc>=0
            # attention(qc) interleaves the A-chains of qc+1 and the
            # projection of qc-1 as PE filler units.
            xt0 = emit_xt_dma(0)
            nc.sync.dma_start(out=wqk_t[:, 0:3, :], in_=wqk[:, 0:3, :])
            nc.sync.dma_start(out=wqk_t[:, 3:6, :], in_=wqk[:, 3:6, :])
            nc.sync.dma_start(
                out=msk_sb, in_=msk.rearrange("p (h w) -> p h w", w=128)
            )
            if with_bias:
                nc.sync.dma_start(out=bqk_bf, in_=bqk[:, :])
                nc.sync.dma_start(out=bv_bf, in_=bv[:, :])
            for tt4 in range(4):
                emit_v_unit(0, xt0, tt4)
            for ct in range(6):
                emit_qk_unit(0, xt0, ct)
            nc.sync.dma_start(out=wp_t, in_=wp[:, :, :])

            pend_prev = []
            xt_next = None
            for qc in range(TC):
                if qc < TC - 1:
                    xt_next = emit_xt_dma(qc + 1)
                    xt_cap = xt_next

                    def mk_v(tt4, xt_cap=xt_cap, q1=qc + 1):
                        return lambda: emit_v_unit(q1, xt_cap, tt4)

                    def mk_qk(ct, xt_cap=xt_cap, q1=qc + 1):
                        return lambda: emit_qk_unit(q1, xt_cap, ct)

                    aunits = [mk_v(tt4) for tt4 in range(4)] + [
                        mk_qk(ct) for ct in range(6)
                    ]
                else:
                    aunits = []
                if qc >= 1:
                    punits = [
                        (lambda tt=tt: emit_proj_unit(qc - 1, tt))
                        for tt in range((qc - 1) * 4, (qc - 1) * 4 + 4)
                    ]
                else:
                    punits = []
                # filler order: two A-units, then the previous chunk's
                # deferred norm finish (its den0 DMA needs latency headroom,
                # but it must precede the proj units), then alternate the
                # remaining A-units with the proj units
                fillers = aunits[:2] + list(pend_prev)
                rest_a = aunits[2:]
                ai, pi = 0, 0
                for i in range(len(rest_a) + len(punits)):
                    if ai < len(rest_a) and (pi >= len(punits) or i % 2 == 0):
                        fillers.append(rest_a[ai])
                        ai += 1
                    else:
                        fillers.append(punits[pi])
                        pi += 1
                last_den, pend_prev = emit_attention(qc, fillers)
            # tail norm chains + final projection are emitted inside the last
            # chunk's attention (quarter-pipelined per diagonal block)

    nc.finalize()
    return nc


def _pv(nc, v_sb, yT_a, yT_b, hp, kt, E, f0, n_kt):
    a = 2 * hp
    nc.tensor.matmul(
        yT_a[:, f0:512],
        v_sb[kt][:, a * 65 : (a + 1) * 65],
        E[:, f0:512],
        start=(kt == 0),
        stop=(kt == n_kt - 1),
    )
    nc.tensor.matmul(
        yT_b[:, f0:512],
        v_sb[kt][:, (a + 1) * 65 : (a + 2) * 65],
        E[:, 512 + f0 : 1024],
        start=(kt == 0),
        stop=(kt == n_kt - 1),
    )


def _get_nc(with_bias: bool):
    if with_bias not in _nc_cache:
        _nc_cache[with_bias] = _build(with_bias)
    return _nc_cache[with_bias]


def kernel(x, W_attn, b_attn, W_proj, b_proj, _run_kwargs=None):
    x = np.ascontiguousarray(np.asarray(x, dtype=np.float32))
    W_attn = np.ascontiguousarray(np.asarray(W_attn, dtype=np.float32))
    b_attn = np.ascontiguousarray(np.asarray(b_attn, dtype=np.float32))
    W_proj = np.ascontiguousarray(np.asarray(W_proj, dtype=np.float32))
    b_proj = np.ascontiguousarray(np.asarray(b_proj, dtype=np.float32))

    with_bias = bool(np.any(b_attn))
    nc = _get_nc(with_bias)

    bf = ml_dtypes.bfloat16

    def _ccp(w):
        # [768, n] -> [128, 6, n]: partition-major, contiguous per partition
        return np.ascontiguousarray(
            w.reshape(CC, 128, -1).transpose(1, 0, 2).astype(bf)
        )

    # x^T per batch as [qc, p, cc, t']: contiguous 6 KB per-partition lines
    xt_by_b = [
        np.ascontiguousarray(
            x[b].T.reshape(CC, 128, TC, 512).transpose(2, 1, 0, 3).astype(bf)
        )
        for b in range(B)
    ]
    tri = np.triu(np.ones((128, 128), dtype=np.float32)).astype(bf)
    trimask = np.ascontiguousarray(np.concatenate([tri, tri], axis=1))
    in_maps = []
    for c in range(8):
        b = c // 2
        hg = c % 2
        cs = slice(hg * 384, (hg + 1) * 384)
        wq = W_attn[:, 0:768][:, cs]
        wk = W_attn[:, 768:1536][:, cs]
        wvs = W_attn[:, 1536:2304][:, cs]
        wps = W_proj[cs, :]
        m = {
            "xt": xt_by_b[b],
            "msk": trimask,
            "wqk": _ccp(np.concatenate([wq, wk], axis=1)),
            "wv": _ccp(wvs),
            "wp": np.ascontiguousarray(
                wps.reshape(3, 128, 768).transpose(1, 0, 2).astype(bf)
            ),
        }
        if with_bias:
            m["bqk"] = np.ascontiguousarray(
                np.concatenate([b_attn[0:768][cs], b_attn[768:1536][cs]]).astype(bf)
            )[None, :]
            m["bv"] = np.ascontiguousarray(b_attn[1536:2304][cs].astype(bf))[None, :]
        in_maps.append(m)

    kwargs = _run_kwargs or {}
    res = run_bass_kernel_spmd(nc, in_maps, core_ids=list(range(8)), **kwargs)

    y = np.empty((B, T, C), dtype=np.float32)
    for b in range(B):
        y[b] = res.results[2 * b]["out"] + res.results[2 * b + 1]["out"]
    y += b_proj[None, None, :]
    if kwargs:
        kernel.last_result = res
    return y


# revision 65
# speedup vs baseline: 1.0568x; 1.0345x over previous
"""Causal self-attention (B=4, T=2048, C=768, 12 heads) on 8 trn2 NeuronCores.

Sharding: core c handles batch b = c//2 and head-group hg = c%2 (6 heads each).
Each core computes its 6 heads end-to-end plus its slice of the output
projection; the two head-group partial projections per batch are summed on the
host (one 6 MB add per batch).

Per-core pipeline (matmuls in bf16 with fp32 PSUM accumulation; softmax fp32):
  - x tiles cast to bf16 on host, transposed to xT; first chunk's DMA split
    per t-tile so the V matmuls start as soon as possible
  - qT/kT = W_qk^T x^T via PE (heads pair-packed into 128-partition tiles)
  - V in natural [t, hd] layout via PE with xT as the stationary operand,
    with a ones column appended per head for the softmax denominator
  - S^T[k,q] = K Q^T per head, two heads per PE slot via tile_position row
    packing (contraction dim = hd = 64); diagonal blocks compute only the
    live column range
  - exp on ACT straight out of PSUM (two heads per op) -> bf16 E; causal
    zero-fill via a host-built triangular bf16 mask multiplied on DVE
  - PV accumulation in PSUM over k tiles (M=65: 64 value dims + denominator)
  - normalization: den + raw-y staged out of PSUM immediately, den row DMA'd
    to partition 0, DVE reciprocal, GpSimd partition_broadcast, muls; the
    bcast+muls of each head pair are deferred one pair

Scheduling (v2): the exp load is strongly back-weighted (the last q-chunk has
5x the exp columns of the first) while its attention matmul work is tiny, so
phase-sequential emission starves either PE or ACT. Instead:
  - the A-phase (QKV) chains of chunk qc+1 and the projection of chunk qc-1
    are interleaved as PE filler UNITS between attention k-blocks (one eager
    filler at each head-pair boundary where S waits on the exp backlog);
  - A-phase PSUM->SBUF staging rides on ACT for early chunks (idle there)
    and on DVE for chunk 3 (ACT is exp-saturated by then);
  - DRAM inputs are host-prearranged partition-major so DMAs move multi-KB
    contiguous lines; wv/x chunk 0 are split so the first chains start early;
  - the final head pair is normalized per output QUARTER as each diagonal
    block's PV lands (quarter j is final after diag block j), reading raw y
    straight from PSUM, with the final projection t-tiles pipelined behind
    the quarter chains -- no keep-warm bridge needed at the tail.
"""

import ml_dtypes
import numpy as np

import concourse.bacc as bacc
import concourse.mybir as mybir
import concourse.tile as tile
from concourse.bass_utils import run_bass_kernel_spmd

F32 = mybir.dt.float32
BF16 = mybir.dt.bfloat16
AF = mybir.ActivationFunctionType

B, T, C = 4, 2048, 768
NH, HD = 12, 64
TC = 4  # t-chunks of 512
CC = 6  # contraction chunks of 128 over C
N_TT = 16  # t tiles of 128

_nc_cache = {}


def _build(with_bias: bool):
    nc = bacc.Bacc(None, target_bir_lowering=False)
    # host-prearranged layouts: partition dim first, contiguous per-partition
    # lines so the DRAM->SBUF DMAs move KB-sized packets
    xt_d = nc.dram_tensor("xt", [TC, 128, CC, 512], BF16, kind="ExternalInput")
    wqk = nc.dram_tensor("wqk", [128, CC, 768], BF16, kind="ExternalInput")
    wv = nc.dram_tensor("wv", [128, CC, 384], BF16, kind="ExternalInput")
    wp = nc.dram_tensor("wp", [128, 3, 768], BF16, kind="ExternalInput")
    msk = nc.dram_tensor("msk", [128, 256], BF16, kind="ExternalInput")
    if with_bias:
        bqk = nc.dram_tensor("bqk", [1, 768], BF16, kind="ExternalInput")
        bv = nc.dram_tensor("bv", [1, 384], BF16, kind="ExternalInput")
    out = nc.dram_tensor("out", [T, C], F32, kind="ExternalOutput")

    with tile.TileContext(nc) as tc, nc.allow_low_precision(
        reason="bf16 matmul operands are intentional"
    ):
        with (
            tc.tile_pool(name="const", bufs=1) as const,
            tc.tile_pool(name="xt", bufs=2) as xtp,
            tc.tile_pool(name="big", bufs=1) as big,
            tc.tile_pool(name="E", bufs=4) as epool,
            tc.tile_pool(name="norm", bufs=2) as npool,
            tc.tile_pool(name="ost", bufs=3) as opool,
            tc.tile_pool(name="ps", bufs=1, space="PSUM") as ps,
        ):
            # ---------------- constants ----------------
            # wdum memset first: it gates the PE warmup matmuls
            wdum = const.tile([128, 512], BF16, name="wdum")
            nc.vector.memset(wdum.bitcast(F32)[:, 0:256], 0.0)

            onecol_f = const.tile([128, 8], F32)
            nc.vector.memset(onecol_f, 1.0)
            onecol_bf = const.tile([128, 8], BF16)
            nc.vector.tensor_copy(onecol_bf, onecol_f)

            # warm the ACT exp table while DMAs run
            warm_f = const.tile([1, 8], F32)
            nc.scalar.activation(warm_f, onecol_f[0:1, :], AF.Exp, scale=1.0)

            if with_bias:
                ones_f = const.tile([1, 512], F32)
                nc.vector.memset(ones_f, 1.0)
                ones_bf = const.tile([1, 512], BF16)
                nc.vector.tensor_copy(ones_bf, ones_f)

            ones64_f = const.tile([1, 64], F32)
            nc.vector.memset(ones64_f, 1.0)

            # PE warmup: dummy matmuls with no input deps keep the HAM
            # activity monitor busy while the first DMAs land; long enough
            # that the clock gate opens (~3.4us) BEFORE the first real A(0)
            # chains, which otherwise run at half clock
            wps = ps.tile([128, 512], F32, tag="mm", bufs=2, name="warmps")
            for _ in range(14):
                nc.tensor.matmul(wps, wdum[:, 0:128], wdum, start=True, stop=True)
            warm_sb = const.tile([1, 8], F32, name="warmsb")
            nc.vector.tensor_copy(warm_sb, wps[0:1, 0:8])

            # ---------------- weights (bf16, host-prepared) ----------------
            # wv split per cc-pair so the first V chain starts after ~1/3
            wv_t = const.tile([128, CC, 384], BF16, name="wv_t")
            for ccp in range(3):
                nc.sync.dma_start(
                    out=wv_t[:, 2 * ccp : 2 * ccp + 2, :],
                    in_=wv[:, 2 * ccp : 2 * ccp + 2, :],
                )
            msk_sb = const.tile([128, 2, 128], BF16, name="msk_sb")
            wv_bf = [wv_t[:, cc, :] for cc in range(CC)]
            wqk_t = const.tile([128, CC, 768], BF16, name="wqk_t")
            wp_t = const.tile([128, 3, 768], BF16, name="wp_t")
            if with_bias:
                bqk_bf = const.tile([1, 768], BF16)
                bv_bf = const.tile([1, 384], BF16)

            # persistent big tiles
            qkT = [big.tile([128, T], BF16, name=f"qkT{ct}") for ct in range(6)]
            v_sb = [big.tile([128, 390], BF16, name=f"v{tt}") for tt in range(N_TT)]
            yT3 = [big.tile([128, T], BF16, name=f"yT{hp}") for hp in range(3)]

            # the denominator ones-columns of v_sb are constant: prefill once
            for tt in range(N_TT):
                vv = v_sb[tt].rearrange("p (h w) -> p h w", w=65)
                nc.vector.tensor_copy(vv[:, :, 64], onecol_bf[:, 0:6])

            def emit_xt_dma(qc):
                """Prefetch the x chunk for chunk qc; returns the tile."""
                xt_t = xtp.tile([128, CC, 512], BF16, tag="xt", name=f"xt_{qc}")
                if qc == 0:
                    # split the first chunk's DMA per contraction chunk so the
                    # first V/qk chains can start as soon as cc=0 lands
                    for cc in range(CC):
                        nc.sync.dma_start(
                            out=xt_t[:, cc, :], in_=xt_d[qc, :, cc, :]
                        )
                else:
                    nc.sync.dma_start(out=xt_t, in_=xt_d[qc, :, :, :])
                return xt_t

            def emit_v_unit(qc, xt_t, tt4):
                """V (+denominator ones) for one t-tile of chunk qc."""
                tt = qc * 4 + tt4
                xt_tiles = [xt_t[:, cc, :] for cc in range(CC)]
                v_ps = ps.tile([128, 384], F32, tag="mm", bufs=2, name=f"vps{tt}")
                for cc in range(CC):
                    nc.tensor.matmul(
                        v_ps,
                        xt_tiles[cc][:, tt4 * 128 : (tt4 + 1) * 128],
                        wv_bf[cc],
                        start=(cc == 0),
                        stop=(cc == CC - 1 and not with_bias),
                    )
                if with_bias:
                    nc.tensor.matmul(
                        v_ps, ones_bf[:, 0:128], bv_bf, start=False, stop=True
                    )
                vv = v_sb[tt].rearrange("p (h w) -> p h w", w=65)
                # early chunks' staging rides on ACT (idle then); chunk 3's
                # lands mid/late attention where ACT is exp-saturated -> DVE
                if qc >= 3:
                    nc.vector.tensor_copy(
                        vv[:, :, 0:64], v_ps.rearrange("p (h w) -> p h w", w=64)
                    )
                else:
                    nc.scalar.copy(
                        vv[:, :, 0:64], v_ps.rearrange("p (h w) -> p h w", w=64)
                    )

            def emit_qk_unit(qc, xt_t, ct):
                """One 128-col slice of qT/kT for chunk qc."""
                xt_tiles = [xt_t[:, cc, :] for cc in range(CC)]
                wqk_bf = [wqk_t[:, cc, :] for cc in range(CC)]
                qk_ps = ps.tile(
                    [128, 512], F32, tag="mm", bufs=2, name=f"qkps{qc}_{ct}"
                )
                for cc in range(CC):
                    nc.tensor.matmul(
                        qk_ps,
                        wqk_bf[cc][:, ct * 128 : (ct + 1) * 128],
                        xt_tiles[cc],
                        start=(cc == 0),
                        stop=(cc == CC - 1 and not with_bias),
                    )
                if with_bias:
                    nc.tensor.matmul(
                        qk_ps,
                        bqk_bf[:, ct * 128 : (ct + 1) * 128],
                        ones_bf,
                        start=False,
                        stop=True,
                    )
                if qc >= 3:
                    nc.vector.tensor_copy(
                        qkT[ct][:, qc * 512 : (qc + 1) * 512], qk_ps
                    )
                else:
                    nc.scalar.copy(qkT[ct][:, qc * 512 : (qc + 1) * 512], qk_ps)

            def emit_proj_unit(qc, tt):
                """Output projection + store for one t-tile of chunk qc."""
                t_sl = slice(tt * 128, (tt + 1) * 128)
                ostage = opool.tile([128, 768], F32, tag="ost")
                for half in range(2):
                    pp = ps.tile(
                        [128, 384], F32, tag="mm", bufs=2, name=f"pj{tt}_{half}"
                    )
                    for hp in range(3):
                        nc.tensor.matmul(
                            pp,
                            yT3[hp][:, t_sl],
                            wp_t[:, hp, half * 384 : (half + 1) * 384],
                            start=(hp == 0),
                            stop=(hp == 2),
                        )
                    nc.vector.tensor_copy(
                        ostage[:, half * 384 : (half + 1) * 384], pp
                    )
                nc.sync.dma_start(out=out[t_sl, :], in_=ostage)

            def emit_attention(qc, fillers):
                """Attention for chunk qc with PE filler units interleaved
                between k-blocks. Returns (last_yst, pending_finishes)."""
                q_sl = slice(qc * 512, (qc + 1) * 512)
                n_kt = 4 * qc + 4
                nblocks = 3 * n_kt
                nfill = len(fillers)
                bi = 0
                fi = 0

                def tick():
                    # advance the filler schedule after each k-block
                    nonlocal bi, fi
                    bi += 1
                    while fi < nfill and fi * nblocks < bi * nfill:
                        fillers[fi]()
                        fi += 1

                tail_quarters = []

                def _tail_quarter_p1(j, yT_a, yT_b):
                    """Phase 1 of the final head pair's quarter-j norm chain
                    (den row staging + DMA to partition 0). Emitted right
                    after diag block j's PV: only cheap wait-free DVE ops, so
                    the remaining attention blocks are not stalled. Phase 2
                    (DMA-dependent) is deferred past the kt loop."""
                    qsl_j = slice(j * 128, (j + 1) * 128)
                    denq = npool.tile([128, 256], F32, tag="denq", bufs=4)
                    nc.vector.tensor_copy(denq[64:65, 0:128], yT_a[64:65, qsl_j])
                    nc.vector.tensor_copy(denq[64:65, 128:256], yT_b[64:65, qsl_j])
                    d0q = npool.tile([1, 512], F32, tag="d0q", bufs=4)
                    nc.sync.dma_start(out=d0q[0:1, 0:256], in_=denq[64:65, :])
                    tail_quarters.append((j, d0q, yT_a, yT_b))

                def _tail_quarter_p2(j, d0q, yT_a, yT_b):
                    """Phase 2: reciprocal, broadcast, PSUM-direct muls (all
                    base-0 operands), partition-shift DMA. The projection
                    t-tiles are emitted separately so the PE queue order
                    keeps all attention matmuls first."""
                    qsl_j = slice(j * 128, (j + 1) * 128)
                    q_abs = slice(qc * 512 + j * 128, qc * 512 + (j + 1) * 128)
                    nc.vector.reciprocal_approx_fast(
                        d0q[0:1, 256:512], d0q[0:1, 0:256]
                    )
                    recq = npool.tile([64, 256], F32, tag="recq", bufs=4)
                    nc.gpsimd.partition_broadcast(
                        recq, d0q[0:1, 256:512], channels=64
                    )
                    nc.vector.tensor_mul(
                        yT3[2][0:64, q_abs], yT_a[0:64, qsl_j], recq[:, 0:128]
                    )
                    ytq = npool.tile([64, 128], BF16, tag="ytq", bufs=4)
                    nc.vector.tensor_mul(ytq, yT_b[0:64, qsl_j], recq[:, 128:256])
                    nc.sync.dma_start(out=yT3[2][64:128, q_abs], in_=ytq)

                pending = []
                last_den = None
                for hp in range(3):
                    if fi < nfill:
                        # one eager filler at each head-pair boundary: the new
                        # pair's first S waits on the previous pair's exp
                        # backlog, so give PE something to chew on
                        fillers[fi]()
                        fi += 1
                    yT_a = ps.tile([65, 512], F32, tag="yT", bufs=2, name=f"ya{qc}_{hp}")
                    yT_b = ps.tile([65, 512], F32, tag="yT", bufs=2, name=f"yb{qc}_{hp}")
                    e_hist = []
                    for kt in range(n_kt):
                        k_sl = slice(kt * 128, (kt + 1) * 128)
                        m = kt - 4 * qc
                        diag = m >= 0
                        w = 512 - 128 * max(m, 0)  # live column range
                        f0 = 512 - w
                        psS = ps.tile(
                            [128, 1024], F32, tag="S", bufs=2, name=f"s{qc}_{hp}_{kt}"
                        )
                        nc.tensor.matmul(
                            psS[:, f0:512],
                            qkT[3 + hp][0:64, k_sl],
                            qkT[hp][0:64, qc * 512 + f0 : (qc + 1) * 512],
                            start=True,
                            stop=True,
                            tile_position=(0, 0),
                        )
                        nc.tensor.matmul(
                            psS[:, 512 + f0 : 1024],
                            qkT[3 + hp][64:128, k_sl],
                            qkT[hp][64:128, qc * 512 + f0 : (qc + 1) * 512],
                            start=True,
                            stop=True,
                            tile_position=(64, 0),
                        )
                        E = epool.tile(
                            [128, 1024], BF16, tag="E", bufs=8, name=f"e{qc}_{hp}_{kt}"
                        )
                        psv = psS.rearrange("p (h w) -> p h w", w=512)
                        ev = E.rearrange("p (h w) -> p h w", w=512)
                        nc.scalar.activation(
                            ev[:, :, f0:512], psv[:, :, f0:512], AF.Exp, scale=0.125
                        )
                        if diag:
                            # causal zero-fill via host-built triangular mask
                            # (DVE: GpSimd's dispatch latency stalls the
                            # E->PV chain even though it has idle capacity)
                            nc.vector.tensor_mul(
                                ev[:, :, f0 : f0 + 128],
                                ev[:, :, f0 : f0 + 128],
                                msk_sb,
                            )
                        e_hist.append((kt, E, f0))
                        tail_pair = qc == TC - 1 and hp == 2
                        if tail_pair and kt >= 4 * qc:
                            # tail head pair, diagonal region: drain PV
                            # eagerly; quarter j of the output columns is
                            # final once diag block m=j has accumulated, so
                            # its normalization chain and its projection
                            # t-tile can start while attention still runs
                            while e_hist:
                                _pv(nc, v_sb, yT_a, yT_b, hp, *e_hist.pop(0), n_kt)
                            if pending:
                                pending.pop(0)()
                            _tail_quarter_p1(kt - 4 * qc, yT_a, yT_b)
                        elif len(e_hist) > 3:
                            # drain two blocks at once: fewer S<->PV
                            # transitions means fewer exposed PV weight loads
                            _pv(nc, v_sb, yT_a, yT_b, hp, *e_hist.pop(0), n_kt)
                            _pv(nc, v_sb, yT_a, yT_b, hp, *e_hist.pop(0), n_kt)
                        tick()
                    if qc == TC - 1 and hp == 2:
                        # all attention matmuls are now emitted; run the
                        # quarter chains one ahead of the projections so
                        # proj(j+1) never waits on its chain
                        _tail_quarter_p2(*tail_quarters.pop(0))
                        for j in range(4):
                            if tail_quarters:
                                _tail_quarter_p2(*tail_quarters.pop(0))
                            emit_proj_unit(qc, 4 * qc + j)
                        continue
                    while e_hist:
                        _pv(nc, v_sb, yT_a, yT_b, hp, *e_hist.pop(0), n_kt)

                    # stage denominator + raw y out of PSUM immediately (frees
                    # the yT banks); den row DMA'd to partition 0 (DVE recip
                    # ucode requires base 0). The broadcast + muls are
                    # deferred one head pair.
                    den = npool.tile(
                        [128, 1024], F32, tag="den", bufs=2, name=f"dn{qc}_{hp}"
                    )
                    nc.vector.tensor_copy(den[64:65, 0:512], yT_a[64:65, :])
                    nc.vector.tensor_copy(den[64:65, 512:1024], yT_b[64:65, :])
                    last_den = den
                    yst = npool.tile(
                        [64, 1024], BF16, tag="yst", bufs=3, name=f"ys{qc}_{hp}"
                    )
                    nc.vector.tensor_copy(yst[:, 0:512], yT_a[0:64, :])
                    nc.vector.tensor_copy(yst[:, 512:1024], yT_b[0:64, :])
                    den0 = npool.tile(
                        [1, 2048], F32, tag="den0", bufs=3, name=f"d0{qc}_{hp}"
                    )
                    nc.sync.dma_start(out=den0[0:1, 0:1024], in_=den[64:65, :])

                    def finish(hp=hp, yst=yst, den0=den0):
                        nc.vector.reciprocal_approx_fast(
                            den0[0:1, 1024:2048], den0[0:1, 0:1024]
                        )
                        rec = npool.tile(
                            [64, 1024], F32, tag="rec", bufs=3, name=f"rc{qc}_{hp}"
                        )
                        nc.gpsimd.partition_broadcast(
                            rec, den0[0:1, 1024:2048], channels=64
                        )
                        nc.vector.tensor_mul(
                            yT3[hp][0:64, q_sl], yst[:, 0:512], rec[:, 0:512]
                        )
                        ytmp = npool.tile(
                            [64, 512], BF16, tag="ytmp", bufs=2, name=f"yt{qc}_{hp}"
                        )
                        nc.vector.tensor_mul(ytmp, yst[:, 512:1024], rec[:, 512:1024])
                        # partition shift 0:64 -> 64:128 via SBUF->SBUF DMA
                        nc.sync.dma_start(out=yT3[hp][64:128, q_sl], in_=ytmp)

                    if pending:
                        pending.pop(0)()
                    pending.append(finish)
                while fi < nfill:
                    fillers[fi]()
                    fi += 1
                return last_den, pending

            # ---------------- main schedule ----------------
            # A(0) emitted directly (nothing to interleave with); for qc>=0
            # attention(qc) interleaves the A-chains of qc+1 and the
            # projection of qc-1 as PE filler units.
            xt0 = emit_xt_dma(0)
            nc.sync.dma_start(out=wqk_t[:, 0:3, :], in_=wqk[:, 0:3, :])
            nc.sync.dma_start(out=wqk_t[:, 3:6, :], in_=wqk[:, 3:6, :])
            nc.sync.dma_start(
                out=msk_sb, in_=msk.rearrange("p (h w) -> p h w", w=128)
            )
            if with_bias:
                nc.sync.dma_start(out=bqk_bf, in_=bqk[:, :])
                nc.sync.dma_start(out=bv_bf, in_=bv[:, :])
            for tt4 in range(4):
                emit_v_unit(0, xt0, tt4)
            for ct in range(6):
                emit_qk_unit(0, xt0, ct)
            nc.sync.dma_start(out=wp_t, in_=wp[:, :, :])

            pend_prev = []
            xt_next = None
            for qc in range(TC):
                if qc < TC - 1:
                    xt_next = emit_xt_dma(qc + 1)
                    xt_cap = xt_next

                    def mk_v(tt4, xt_cap=xt_cap, q1=qc + 1):
                        return lambda: emit_v_unit(q1, xt_cap, tt4)

                    def mk_qk(ct, xt_cap=xt_cap, q1=qc + 1):
                        return lambda: emit_qk_unit(q1, xt_cap, ct)

                    aunits = [mk_v(tt4) for tt4 in range(4)] + [
                        mk_qk(ct) for ct in range(6)
                    ]
                else:
                    aunits = []
                if qc >= 1:
                    punits = [
                        (lambda tt=tt: emit_proj_unit(qc - 1, tt))
                        for tt in range((qc - 1) * 4, (qc - 1) * 4 + 4)
                    ]
                else:
                    punits = []
                # filler order: two A-units, then the previous chunk's
                # deferred norm finish (its den0 DMA needs latency headroom,
                # but it must precede the proj units), then alternate the
                # remaining A-units with the proj units
                fillers = aunits[:2] + list(pend_prev)
                rest_a = aunits[2:]
                ai, pi = 0, 0
                for i in range(len(rest_a) + len(punits)):
                    if ai < len(rest_a) and (pi >= len(punits) or i % 2 == 0):
                        fillers.append(rest_a[ai])
                        ai += 1
                    else:
                        fillers.append(punits[pi])
                        pi += 1
                last_den, pend_prev = emit_attention(qc, fillers)
            # tail norm chains + final projection are emitted inside the last
            # chunk's attention (quarter-pipelined per diagonal block)

    nc.finalize()
    return nc


def _pv(nc, v_sb, yT_a, yT_b, hp, kt, E, f0, n_kt):
    a = 2 * hp
    nc.tensor.matmul(
        yT_a[:, f0:512],
        v_sb[kt][:, a * 65 : (a + 1) * 65],
        E[:, f0:512],
        start=(kt == 0),
        stop=(kt == n_kt - 1),
    )
    nc.tensor.matmul(
        yT_b[:, f0:512],
        v_sb[kt][:, (a + 1) * 65 : (a + 2) * 65],
        E[:, 512 + f0 : 1024],
        start=(kt == 0),
        stop=(kt == n_kt - 1),
    )


def _get_nc(with_bias: bool):
    if with_bias not in _nc_cache:
        _nc_cache[with_bias] = _build(with_bias)
    return _nc_cache[with_bias]


def kernel(x, W_attn, b_attn, W_proj, b_proj, _run_kwargs=None):
    x = np.ascontiguousarray(np.asarray(x, dtype=np.float32))
    W_attn = np.ascontiguousarray(np.asarray(W_attn, dtype=np.float32))
    b_attn = np.ascontiguousarray(np.asarray(b_attn, dtype=np.float32))
    W_proj = np.ascontiguousarray(np.asarray(W_proj, dtype=np.float32))
    b_proj = np.ascontiguousarray(np.asarray(b_proj, dtype=np.float32))

    with_bias = bool(np.any(b_attn))
    nc = _get_nc(with_bias)

    bf = ml_dtypes.bfloat16

    def _ccp(w):
        # [768, n] -> [128, 6, n]: partition-major, contiguous per partition
        return np.ascontiguousarray(
            w.reshape(CC, 128, -1).transpose(1, 0, 2).astype(bf)
        )

    # x^T per batch as [qc, p, cc, t']: contiguous 6 KB per-partition lines
    xt_by_b = [
        np.ascontiguousarray(
            x[b].T.reshape(CC, 128, TC, 512).transpose(2, 1, 0, 3).astype(bf)
        )
        for b in range(B)
    ]
    tri = np.triu(np.ones((128, 128), dtype=np.float32)).astype(bf)
    trimask = np.ascontiguousarray(np.concatenate([tri, tri], axis=1))
    in_maps = []
    for c in range(8):
        b = c // 2
        hg = c % 2
        cs = slice(hg * 384, (hg + 1) * 384)
        wq = W_attn[:, 0:768][:, cs]
        wk = W_attn[:, 768:1536][:, cs]
        wvs = W_attn[:, 1536:2304][:, cs]
        wps = W_proj[cs, :]
        m = {
            "xt": xt_by_b[b],
            "msk": trimask,
            "wqk": _ccp(np.concatenate([wq, wk], axis=1)),
            "wv": _ccp(wvs),
            "wp": np.ascontiguousarray(
                wps.reshape(3, 128, 768).transpose(1, 0, 2).astype(bf)
            ),
        }
        if with_bias:
            m["bqk"] = np.ascontiguousarray(
                np.concatenate([b_attn[0:768][cs], b_attn[768:1536][cs]]).astype(bf)
            )[None, :]
            m["bv"] = np.ascontiguousarray(b_attn[1536:2304][cs].astype(bf))[None, :]
        in_maps.append(m)

    kwargs = _run_kwargs or {}
    res = run_bass_kernel_spmd(nc, in_maps, core_ids=list(range(8)), **kwargs)

    y = np.empty((B, T, C), dtype=np.float32)
    for b in range(B):
        y[b] = res.results[2 * b]["out"] + res.results[2 * b + 1]["out"]
    y += b_proj[None, None, :]
    if kwargs:
        kernel.last_result = res
    return y
